# revision 43
# baseline (speedup 1.0000x reference)
"""CrossAttention Trainium2 kernel (mask-compacted).

Problem (hardcoded): B=8, T=256, S=4096, E=512, KV=768, H=8, D=64.
Sharding: data-parallel over B — one batch per NeuronCore (8 cores).

Key idea vs v1: ~50% of keys are masked (key_padding_mask True = ignore)
and masked keys provably don't contribute to the output (softmax weight
exactly 0 via the m01 fold into V'). So the host compacts each batch's
context to only the kept keys, padded to a common S_pad (multiple of
128, ~2176 for the harness seed). All S-proportional device work
(KV-proj, scores, exp, PV) drops by ~1.9x. Padding rows have ctx=0 =>
k=0 => score=0 => exp=1, but m01=0 zeroes their V' rows and ones-col so
they add 0 to both numerator and denominator.

Per-core dataflow (one batch, layouts staged host-side, bf16 unless noted):
    ctxT  [768, S_pad]  = compacted context[b].T
    xT    [512, 256], wqT (scale folded), wkvT [768,1024], woT, bo_r
    m01   [128, N_SC] f32 = 1.0 kept / 0.0 pad   (s = sc*128 + p)
  device:
    QT    = wqT.T @ xT -> [512c, 256t]
    KT    = wkvT[:, :512].T @ ctxT -> [512c, S_pad]  (c-major, 4 head pairs)
    V'    = ctxT.T @ wkvT[:, 512:] -> per-sc [128 s, 8h*65] * m01
    scoresT[s,t] per head: KT head slices as lhsT (K=64, head pair packed
            into PE row groups 0:64/64:128 -> concurrent row-tiled MMs)
    expsT = Exp(scoresT) on ACT; PV = V'_h @ expsT -> [65,256], row 64 =
            softmax denominator; accumulated per 4-sc group in PSUM then
            DVE-added into SBUF pvacc.
    norm  = reciprocal_approx_fast(denoms) broadcast via K=1 matmul;
            OT = PV * recip ; outT = woT.T @ OT + bo.

Schedule: software pipeline over 512-col ctx groups g: iteration g issues
scores(g-1, kc) / kv-proj(g, kc) interleaved so the ACT exp of group g-1
runs under the kv-proj matmuls of group g, then PV(g-1, kc) / V'(g, sc)
interleaved. Tail normalization is incremental per head pair and feeds
an out-proj PSUM accumulation, keeping the PE warm to the end.
"""

import sys

sys.path.insert(0, "/opt/trn_rl_repo")

import numpy as np
import ml_dtypes
from contextlib import ExitStack

import concourse.bass as bass
import concourse.bacc as bacc
import concourse.tile as tile
from concourse import mybir
from concourse import bass_utils

BF16 = mybir.dt.bfloat16
F32 = mybir.dt.float32
F8E4 = mybir.dt.float8e4
NPBF16 = ml_dtypes.bfloat16
NPF8 = ml_dtypes.float8_e4m3fn

B, T, S, E, KV, H, D = 8, 256, 4096, 512, 768, 8, 64
NC_CORES = 8


def _groups(n_sc):
    """Split n_sc 128-wide s-chunks into groups of <=4 (512 ctx cols).

    The remainder group goes FIRST: a tiny group 0 lets the PE start on
    kv-proj as soon as possible (small first ctx DMA), and a full-size
    last group gives the software pipeline real PE work to overlap the
    tail normalization latency with.
    """
    rem = n_sc % 4
    out = []
    sc0 = 0
    if rem:
        out.append((0, rem))
        sc0 = rem
    while sc0 < n_sc:
        out.append((sc0, 4))
        sc0 += 4
    return out


def _build_program(s_pad):
    n_sc = s_pad // 128
    groups = _groups(n_sc)
    n_g = len(groups)

    nc = bacc.Bacc("TRN2", target_bir_lowering=False, debug=False)

    # Host pre-packs every multi-chunk tensor c-side-by-side on 128
    # partitions so each input needs only one or two big DMA transfers —
    # queue postings serialize on completion semaphores, so many small
    # transfers cost ~1.5-2us each in arrival latency.
    ctxp_d = [
        nc.dram_tensor(
            f"ctxp{g}", [128, 6 * 128 * groups[g][1]], BF16, kind="ExternalInput"
        ).ap()
        for g in range(n_g)
    ]
    xp_d = nc.dram_tensor("xp", [128, 4 * T], BF16, kind="ExternalInput").ap()
    m01_d = nc.dram_tensor("m01", [128, n_sc], F32, kind="ExternalInput").ap()
    wqp_d = nc.dram_tensor("wqp", [128, 4 * 512], BF16, kind="ExternalInput").ap()
    wkvp_d = nc.dram_tensor("wkvp", [128, 6 * 512], BF16, kind="ExternalInput").ap()
    wk8p_d = nc.dram_tensor("wk8p", [128, 3 * 1024], F8E4, kind="ExternalInput").ap()
    ctx8p_d = [
        nc.dram_tensor(
            f"ctx8p{g}", [128, 6 * 128 * groups[g][1]], F8E4, kind="ExternalInput"
        ).ap()
        for g in range(n_g)
    ]
    wop_d = nc.dram_tensor("wop", [128, 4 * 512], BF16, kind="ExternalInput").ap()
    bo_d = nc.dram_tensor("bo_r", [128, 4], F32, kind="ExternalInput").ap()
    outT_d = nc.dram_tensor("outT", [4, 128, T], F32, kind="ExternalOutput").ap()

    with tile.TileContext(nc) as tc, ExitStack() as ctx:
        const = ctx.enter_context(tc.tile_pool(name="const", bufs=1))
        work = ctx.enter_context(tc.tile_pool(name="work", bufs=2))
        # PSUM: 3 scores banks + 3 kv-aux banks + 2 PV banks = 8.
        p_sc = ctx.enter_context(tc.tile_pool(name="p_sc", bufs=3, space="PSUM"))
        p_a = ctx.enter_context(tc.tile_pool(name="p_a", bufs=3, space="PSUM"))
        p_pv = ctx.enter_context(tc.tile_pool(name="p_pv", bufs=2, space="PSUM"))

        # ---- static SBUF tensors -------------------------------------------
        ctxp_t = [
            const.tile(
                [128, 6 * 128 * groups[g][1]], BF16, tag=f"ctxp{g}", name=f"ctxp{g}"
            )
            for g in range(n_g)
        ]

        def ctx_slice(g, c, lo, hi):
            w = 128 * groups[g][1]
            return ctxp_t[g][:, c * w + lo : c * w + hi]

        kt_t = [
            const.tile([128, s_pad], BF16, tag=f"kt{kc}", name=f"kt{kc}")
            for kc in range(4)
        ]
        vp_t = [
            const.tile([128, 8 * 65], BF16, tag=f"vp{sc}", name=f"vp{sc}")
            for sc in range(n_sc)
        ]
        qt_t = [
            const.tile([128, T], BF16, tag=f"qt{qc}", name=f"qt{qc}") for qc in range(4)
        ]
        ot_t = [
            const.tile([128, T], BF16, tag=f"ot{cc}", name=f"ot{cc}") for cc in range(4)
        ]
        wqp_t = const.tile([128, 4 * 512], BF16, tag="wqp")
        wkvp_t = const.tile([128, 6 * 512], BF16, tag="wkvp")
        wk8p_t = const.tile([128, 3 * 1024], F8E4, tag="wk8p")
        ctx8p_t = [
            const.tile(
                [128, 6 * 128 * groups[g][1]], F8E4, tag=f"ctx8p{g}", name=f"ctx8p{g}"
            )
            for g in range(n_g)
        ]
        wop_t = const.tile([128, 4 * 512], BF16, tag="wop")
        xp_t = const.tile([128, 4 * T], BF16, tag="xp")

        def wkvv_slice(c):
            return wkvp_t[:, c * 512 : (c + 1) * 512]

        def wk8_slice(r, kc):
            v = wk8p_t[:, r * 1024 : (r + 1) * 1024].rearrange(
                "p (i m) -> p i m", i=2
            )
            return v[:, :, kc * 128 : (kc + 1) * 128]

        def ctx8_slice(g, r):
            w = 128 * groups[g][1]
            return ctx8p_t[g][:, r * 2 * w : (r + 1) * 2 * w].rearrange(
                "p (i s) -> p i s", i=2
            )

        pvacc_t = [
            const.tile([65, T], F32, tag=f"pvacc{h}", name=f"pvacc{h}")
            for h in range(8)
        ]
        den2_t = [
            const.tile([2, T], F32, tag=f"den2_{kc}", name=f"den2_{kc}")
            for kc in range(4)
        ]
        rec2_t = [
            const.tile([2, T], F32, tag=f"rec2_{kc}", name=f"rec2_{kc}")
            for kc in range(4)
        ]
        rech_t = const.tile([1, 8 * T], F32, tag="rech")
        m01_t = const.tile([128, n_sc], F32, tag="m01")
        bo_t = const.tile([128, 4], F32, tag="bo")
        ones8_t = const.tile([128, 8], BF16, tag="ones8")
        ones64_t = const.tile([1, 64], F32, tag="ones64")

        # ---- loads ----------------------------------------------------------
        # 3 DMA queues (sync/SP, gpsimd, scalar/Activation).
        #   sync:   x, wq (Q-proj deps) then later ctx groups
        #   gpsimd: ctx group 0 (tiny) + group 1 then later groups
        #   scalar: wkv (consumed c-outer by group-0 K-part), then cold path
        nc.vector.memset(ones8_t[:], 1.0)
        nc.vector.memset(ones64_t[:], 1.0)
        g0w = 128 * groups[0][1]
        nc.sync.dma_start(xp_t[:], xp_d)
        nc.gpsimd.dma_start(wk8p_t[:], wk8p_d)
        nc.gpsimd.dma_start(ctx8p_t[0][:], ctx8p_d[0])
        nc.sync.dma_start(wqp_t[:], wqp_d)
        nc.scalar.dma_start(wkvp_t[:], wkvp_d)
        nc.gpsimd.dma_start(ctxp_t[0][:], ctxp_d[0])
        nc.gpsimd.dma_start(ctx8p_t[1][:], ctx8p_d[1])
        nc.sync.dma_start(ctxp_t[1][:], ctxp_d[1])
        nc.scalar.dma_start(m01_t[:], m01_d)
        nc.scalar.dma_start(bo_t[:], bo_d)
        for g in range(2, n_g):
            q = [nc.gpsimd, nc.sync, nc.scalar][g % 3]
            q.dma_start(ctx8p_t[g][:], ctx8p_d[g])
            q.dma_start(ctxp_t[g][:], ctxp_d[g])
        nc.scalar.dma_start(wop_t[:], wop_d)

        # ---- HAM warm-up ----------------------------------------------------
        # The PE clock sits at 1.2 GHz until ~3.4us of sustained activity.
        # Real work is DMA-gated for the first ~6us after the preamble, so
        # burn that window on dependency-free dummy matmuls: by the time x/wq
        # land, the PE runs at 2.4 GHz and stays there.
        warm_w = const.tile([128, 128], BF16, tag="warm_w")
        nc.vector.memset(warm_w[:], 0.0)
        wps = p_a.tile([128, 512], F32, tag="a", name="warm_ps")
        for i in range(56):
            nc.tensor.matmul(
                wps[:, 0:128], lhsT=warm_w[:], rhs=warm_w[:], start=True, stop=True
            )

        # ---- Q projection (PE warm-up while ctx/wkv stream) -----------------
        for qc in range(4):
            qps = p_sc.tile([128, 512], F32, tag="sc", name=f"qps{qc}")
            for ec in range(4):
                nc.tensor.matmul(
                    qps[:, 0:T],
                    lhsT=wqp_t[:, ec * 512 + qc * 128 : ec * 512 + (qc + 1) * 128],
                    rhs=xp_t[:, ec * T : (ec + 1) * T],
                    start=(ec == 0),
                    stop=(ec == 3),
                )
            nc.vector.tensor_copy(qt_t[qc][:], qps[:, 0:T])

        # ---- group-0 K-part (fp8 DoubleRow: kv contracted 256/pass) ---------
        for kc in range(4):
            aps0 = p_sc.tile([128, 512], F32, tag="sc", name=f"aps0_{kc}")
            for r in range(3):
                nc.tensor.matmul(
                    aps0[:, 0:g0w],
                    lhsT=wk8_slice(r, kc),
                    rhs=ctx8_slice(0, r)[:, :, 0:g0w],
                    start=(r == 0),
                    stop=(r == 2),
                    perf_mode=mybir.MatmulPerfMode.DoubleRow,
                )
            nc.vector.tensor_copy(kt_t[kc][:, 0:g0w], aps0[:, 0:g0w])

        sc2g = {}
        for gi, (sc0_, nsc_) in enumerate(groups):
            for sc_ in range(sc0_, sc0_ + nsc_):
                sc2g[sc_] = gi

        def vprime(sc):
            """V' for one 128-wide s-chunk: [128 s, 8h*65] with mask folded."""
            g = sc2g[sc]
            off = (sc - groups[g][0]) * 128
            ps = p_a.tile([128, 512], F32, tag="a", name=f"vps{sc}")
            for c in range(6):
                nc.tensor.matmul(
                    ps[:],
                    lhsT=ctx_slice(g, c, off, off + 128),
                    rhs=wkvv_slice(c),
                    start=(c == 0),
                    stop=(c == 5),
                )
            dst = vp_t[sc][:].rearrange("p (h e) -> p h e", e=65)
            nc.vector.tensor_scalar_mul(
                dst[:, :, 0:64],
                ps[:].rearrange("p (h d) -> p h d", d=64),
                m01_t[:, sc : sc + 1],
            )
            nc.vector.tensor_scalar_mul(
                dst[:, :, 64:65],
                ones8_t[:].rearrange("p (h o) -> p h o", o=1),
                m01_t[:, sc : sc + 1],
            )

        kps_live = {}

        def kpart(g, kc, rs):
            """K-projection slice kc for ctx group g (fp8 DoubleRow passes rs)."""
            sc0, nsc = groups[g]
            w = nsc * 128
            if rs[0] == 0:
                kps_live[0] = p_a.tile([128, 512], F32, tag="a", name=f"kps{g}_{kc}")
            ps = kps_live[0]
            for r in rs:
                nc.tensor.matmul(
                    ps[:, 0:w],
                    lhsT=wk8_slice(r, kc),
                    rhs=ctx8_slice(g, r)[:, :, 0:w],
                    start=(r == 0),
                    stop=(r == 2),
                    perf_mode=mybir.MatmulPerfMode.DoubleRow,
                )
            if rs[-1] == 2:
                nc.vector.tensor_copy(
                    kt_t[kc][:, sc0 * 128 : sc0 * 128 + w], ps[:, 0:w]
                )

        def alloc_exp(g, kc):
            e0 = work.tile([128, 1024], BF16, tag="exp", bufs=8, name=f"e0_{g}_{kc}")
            e1 = work.tile([128, 1024], BF16, tag="exp", bufs=8, name=f"e1_{g}_{kc}")
            exps[(g, kc)] = (e0, e1)

        def scores_half(g, kc, jh):
            """ScoresT + exp for head pair kc, group g, j-half jh (0/1).

            Each half is a [128,<=512] psum pair from the 3-deep rotation;
            one ACT Exp per tile keeps ACT inside the PE's per-half budget.
            """
            sc0, nsc = groups[g]
            j0 = jh * 2
            j1 = min(j0 + 2, nsc)
            if j1 <= j0:
                return
            w = (j1 - j0) * 256
            e0, e1 = exps[(g, kc)]
            pe0 = p_sc.tile([128, 512], F32, tag="sc", name=f"pe0_{g}_{kc}_{jh}")
            pe1 = p_sc.tile([128, 512], F32, tag="sc", name=f"pe1_{g}_{kc}_{jh}")
            for j in range(j0, j1):
                sc = sc0 + j
                nc.tensor.matmul(
                    pe0[:, (j - j0) * 256 : (j - j0 + 1) * 256],
                    lhsT=kt_t[kc][0:64, sc * 128 : (sc + 1) * 128],
                    rhs=qt_t[kc][0:64, :],
                    start=True,
                    stop=True,
                )
                nc.tensor.matmul(
                    pe1[:, (j - j0) * 256 : (j - j0 + 1) * 256],
                    lhsT=kt_t[kc][64:128, sc * 128 : (sc + 1) * 128],
                    rhs=qt_t[kc][64:128, :],
                    start=True,
                    stop=True,
                )
            nc.scalar.activation(
                e0[:, j0 * 256 : j0 * 256 + w], pe0[:, 0:w],
                mybir.ActivationFunctionType.Exp,
            )
            nc.scalar.activation(
                e1[:, j0 * 256 : j0 * 256 + w], pe1[:, 0:w],
                mybir.ActivationFunctionType.Exp,
            )

        def pv(g, kc):
            """PV for head pair kc over group g, accumulate into pvacc."""
            sc0, nsc = groups[g]
            e0, e1 = exps.pop((g, kc))
            pvq0 = p_pv.tile([65, T], F32, tag="pv", name=f"pvq0_{g}_{kc}")
            pvq1 = p_pv.tile([65, T], F32, tag="pv", name=f"pvq1_{g}_{kc}")
            for j in range(nsc):
                sc = sc0 + j
                nc.tensor.matmul(
                    pvq0[:],
                    lhsT=vp_t[sc][:, (2 * kc) * 65 : (2 * kc) * 65 + 65],
                    rhs=e0[:, j * 256 : (j + 1) * 256],
                    start=(j == 0),
                    stop=(j == nsc - 1),
                )
                nc.tensor.matmul(
                    pvq1[:],
                    lhsT=vp_t[sc][:, (2 * kc + 1) * 65 : (2 * kc + 1) * 65 + 65],
                    rhs=e1[:, j * 256 : (j + 1) * 256],
                    start=(j == 0),
                    stop=(j == nsc - 1),
                )
            if g == 0:
                nc.vector.tensor_copy(pvacc_t[2 * kc][:], pvq0[:])
                nc.vector.tensor_copy(pvacc_t[2 * kc + 1][:], pvq1[:])
            else:
                nc.vector.tensor_add(pvacc_t[2 * kc][:], pvacc_t[2 * kc][:], pvq0[:])
                nc.vector.tensor_add(
                    pvacc_t[2 * kc + 1][:], pvacc_t[2 * kc + 1][:], pvq1[:]
                )

        exps = {}

        def den_chain(kc):
            """Collect denominators of head pair kc, reciprocal, refold to rech."""
            nc.sync.dma_start(den2_t[kc][0:1, :], pvacc_t[2 * kc][64:65, :])
            nc.gpsimd.dma_start(den2_t[kc][1:2, :], pvacc_t[2 * kc + 1][64:65, :])
            nc.vector.reciprocal_approx_fast(rec2_t[kc][:], den2_t[kc][:])
            nc.sync.dma_start(
                rech_t[0:1, (2 * kc) * T : (2 * kc + 2) * T].rearrange(
                    "p (h t) -> p h t", t=T
                ),
                rec2_t[kc][:],
            )

        def bc_mul(kc):
            """Broadcast 1/den over 64 partitions (K=1 matmul), scale PV -> OT."""
            bc = p_a.tile([128, 512], F32, tag="a", name=f"bc{kc}")
            nc.tensor.matmul(
                bc[0:64, 0:512],
                lhsT=ones64_t[:],
                rhs=rech_t[0:1, (2 * kc) * T : (2 * kc + 2) * T],
                start=True,
                stop=True,
            )
            tmp1 = work.tile([64, T], BF16, tag="otmp", bufs=2, name=f"otmp{kc}")
            nc.vector.tensor_mul(
                tmp1[:], pvacc_t[2 * kc + 1][0:64, :], bc[0:64, T : 2 * T]
            )
            nc.gpsimd.dma_start(ot_t[kc][64:128, :], tmp1[:])
            nc.vector.tensor_mul(
                ot_t[kc][0:64, :], pvacc_t[2 * kc][0:64, :], bc[0:64, 0:T]
            )

        def outproj_all():
            """Out-proj eo-outer: one [128,512] accumulator at a time from
            the 3-deep rotation; bias + output DMA fire per eo."""
            for eo in range(4):
                reg = p_sc.tile([128, 512], F32, tag="sc", name=f"ops{eo}")
                for kc in range(4):
                    nc.tensor.matmul(
                        reg[:, 0:T],
                        lhsT=wop_t[:, kc * 512 + eo * 128 : kc * 512 + (eo + 1) * 128],
                        rhs=ot_t[kc][:],
                        start=(kc == 0),
                        stop=(kc == 3),
                    )
                osb = work.tile([128, T], F32, tag="osb", bufs=4, name=f"osb{eo}")
                nc.vector.tensor_scalar_add(osb[:], reg[:, 0:T], bo_t[:, eo : eo + 1])
                q = nc.sync if eo % 2 == 0 else nc.gpsimd
                q.dma_start(outT_d[eo], osb[:])

        # ---- V'(0) then pipelined groups -----------------------------------
        for sc in range(groups[0][0], groups[0][0] + groups[0][1]):
            vprime(sc)

        # Single-phase slots: scores/kpart/V'/PV interleaved per head pair
        # (PV shifted one slot so its exp is ready) keeps every engine fed
        # at its own rate instead of alternating ACT-paced and ACT-idle
        # phases.
        for g in range(1, n_g):
            for kc in range(4):
                alloc_exp(g - 1, kc)
                scores_half(g - 1, kc, 0)
                kpart(g, kc, [0, 1])
                scores_half(g - 1, kc, 1)
                kpart(g, kc, [2])
                if kc < groups[g][1]:
                    vprime(groups[g][0] + kc)
                if kc >= 1:
                    pv(g - 1, kc - 1)
            pv(g - 1, 3)

        # ---- final group: scores/PV interleaved with the normalization tail.
        gl = n_g - 1
        for kc in range(4):
            alloc_exp(gl, kc)
            scores_half(gl, kc, 0)
            if kc >= 1:
                pv(gl, kc - 1)
                den_chain(kc - 1)
            scores_half(gl, kc, 1)
            if kc >= 2:
                bc_mul(kc - 2)
        pv(gl, 3)
        den_chain(3)
        bc_mul(2)
        bc_mul(3)
        outproj_all()

    nc.compile()
    return nc


_NC_CACHE = {}


def _get_nc(s_pad):
    if s_pad not in _NC_CACHE:
        _NC_CACHE[s_pad] = _build_program(s_pad)
    return _NC_CACHE[s_pad]


def _pack6(mT):
    """[k*128, N] -> [128, k*N] with chunk c at cols c*N."""
    n = mT.shape[1]
    return np.ascontiguousarray(
        mT.reshape(-1, 128, n).transpose(1, 0, 2).reshape(128, -1)
    )


def _prep_in_maps(x, context, key_padding_mask, Wq, Wkv, Wo, bo):
    keep = [np.flatnonzero(~key_padding_mask[b]) for b in range(B)]
    max_keep = max(len(k) for k in keep)
    s_pad = max(128, -(-max_keep // 128) * 128)
    groups = _groups(s_pad // 128)

    wqp = _pack6((np.ascontiguousarray(Wq.T) * np.float32(D**-0.5)).astype(NPBF16))
    wkvT = np.ascontiguousarray(Wkv.T)  # [768, 1024]
    wkvp = _pack6(np.ascontiguousarray(wkvT[:, 512:1024]).astype(NPBF16))
    wk8p = np.ascontiguousarray(
        wkvT[:, 0:512].reshape(3, 2, 128, 512).transpose(2, 0, 1, 3).reshape(128, -1)
    ).astype(NPF8)
    wop = _pack6(np.ascontiguousarray(Wo.T).astype(NPBF16))
    bo_r = np.ascontiguousarray(bo.reshape(4, 128).T).astype(np.float32)
    in_maps = []
    for b in range(B):
        nk = len(keep[b])
        ctxc = np.zeros((s_pad, KV), dtype=np.float32)
        ctxc[:nk] = context[b][keep[b]]
        ctxT = np.ascontiguousarray(ctxc.T).astype(NPBF16)  # [768, s_pad]
        ctx3 = ctxT.reshape(6, 128, s_pad)
        m = np.zeros(s_pad, dtype=np.float32)
        m[:nk] = 1.0
        m01 = np.ascontiguousarray(m.reshape(s_pad // 128, 128).T)
        ctx8 = np.ascontiguousarray(
            ctxT.astype(np.float32)
            .reshape(3, 2, 128, s_pad)
            .transpose(2, 0, 1, 3)
        ).astype(NPF8)  # [128, 3, 2, s_pad]
        im = dict(
            xp=_pack6(np.ascontiguousarray(x[b].T).astype(NPBF16)),
            m01=m01, wqp=wqp, wkvp=wkvp, wk8p=wk8p, wop=wop, bo_r=bo_r,
        )
        for g, (sc0, nsc) in enumerate(groups):
            blk = ctx3[:, :, sc0 * 128 : (sc0 + nsc) * 128]  # [6,128,w]
            im[f"ctxp{g}"] = np.ascontiguousarray(
                blk.transpose(1, 0, 2).reshape(128, -1)
            )
            im[f"ctx8p{g}"] = np.ascontiguousarray(
                ctx8[:, :, :, sc0 * 128 : (sc0 + nsc) * 128].reshape(128, -1)
            )
        in_maps.append(im)
    return in_maps, s_pad


def _run(inputs, trace=False, **kw):
    in_maps, s_pad = _prep_in_maps(**inputs)
    nc = _get_nc(s_pad)
    res = bass_utils.run_bass_kernel_spmd(
        nc, in_maps, core_ids=list(range(NC_CORES)), trace=trace, **kw
    )
    out = np.stack(
        [res.results[b]["outT"].reshape(E, T).T for b in range(B)]
    ).astype(np.float32)
    return out, res


def kernel(**inputs):
    out, _ = _run(inputs, trace=False)
    return out


if __name__ == "__main__":
    rng = np.random.default_rng(0)
    ins = dict(
        x=rng.standard_normal((B, T, E), dtype=np.float32),
        context=rng.standard_normal((B, S, KV), dtype=np.float32),
        key_padding_mask=rng.integers(0, 2, (B, S)).astype(bool),
        Wq=(rng.standard_normal((512, E), dtype=np.float32) * 0.02),
        Wkv=(rng.standard_normal((1024, KV), dtype=np.float32) * 0.02),
        Wo=(rng.standard_normal((E, 512), dtype=np.float32) * 0.02),
        bo=np.zeros(E, dtype=np.float32),
    )
    out = kernel(**ins)
    print("out", out.shape, out.dtype, np.abs(out).mean())


# revision 45
# speedup vs baseline: 1.1422x; 1.1422x over previous
"""CrossAttention Trainium2 kernel (mask-compacted, fp8 K-projection).

Problem (hardcoded): B=8, T=256, S=4096, E=512, KV=768, H=8, D=64.
Sharding: data-parallel over B — one batch per NeuronCore (8 cores).

Key ideas:
  * ~50% of keys are masked (key_padding_mask True = ignore) and masked
    keys provably contribute nothing (softmax weight exactly 0 via the
    m01 fold into V'), so the host compacts each batch's context to the
    kept keys padded to a common S_pad (multiple of 128; ~2176 for the
    harness seed). All S-proportional device work (KV-proj, scores, exp,
    PV) drops ~1.9x. Padding rows: ctx=0 => k=0 => exp(score)=1, but
    m01=0 zeroes their V' rows and ones-col so they add 0 to numerator
    and denominator. Exact math — compaction changes nothing.
  * K-projection runs in fp8-e4m3 with perf_mode=DoubleRow (2 fp8 MACs
    per PE cell per cycle, kv contracted 256/pass => 3 passes instead of
    6). Only the K path is quantized: scores pass through exp and a
    ~2000-key average, so the ~5% per-element fp8 noise lands at ~1.6e-2
    rel_l2 on the output (vs 4.2e-3 all-bf16, gate 2e-2, deterministic).
    V stays bf16 (V noise passes straight through to the output).
  * Host packs every multi-chunk input side-by-side on 128 partitions so
    each tensor ships in 1-2 large DMA transfers (queue postings
    serialize on completion semaphores; many small transfers cost
    ~1.5-2us each in arrival latency).
  * A dependency-free dummy-matmul warmup burns the DMA-gated start
    window so the PE HAM clock is at 2.4 GHz when real work lands.

Per-core dataflow (bf16 unless noted):
    QT    = wqT.T @ xT -> [512c, 256t]          (scale folded into wq)
    KT    = Wk.T @ ctxT (fp8 DoubleRow) -> [512c, S_pad] (4 head pairs)
    V'    = ctxT.T @ Wv -> per-sc [128 s, 8h*65] * m01 (65th col = m01)
    scoresT[s,t] per head: KT head slices as lhsT (K=64, head pair packed
            into PE row groups 0:64/64:128 -> concurrent row-tiled MMs)
    expsT = Exp(scoresT) on ACT; PV = V'_h @ expsT -> [65,256] psum, row
            64 = softmax denominator; per-group PSUM accumulation then
            DVE-added into SBUF pvacc.
    norm  = reciprocal_approx_fast(denoms), broadcast over 64 partitions
            via a K=1 matmul; OT = PV * recip; outT = woT.T @ OT + bo.

Schedule: single-phase software pipeline over 512-col ctx groups g
(remainder group first). Each slot kc of iteration g interleaves
scores(g-1,kc) halves / kpart(g,kc) fp8 passes / V'(g,kc) / PV(g-1,kc-1)
so PE, ACT (exp), DVE (evictions, V' scaling, PV accumulation) and the
DMA queues all stay fed at their own rates; scores psum rotates through
3 single-bank buffers (PSUM budget: 3 scores + 3 kv-aux + 2 PV = 8
banks). The final group interleaves the per-head-pair denominator
chains (psum row -> reciprocal -> rech), the K=1 broadcast matmuls and
OT scaling into the last scores/PV slots, then the out-projection
accumulates per-eo with bias + output DMA fired as each chunk closes.
"""

import sys

sys.path.insert(0, "/opt/trn_rl_repo")

import numpy as np
import ml_dtypes
from contextlib import ExitStack

import concourse.bass as bass
import concourse.bacc as bacc
import concourse.tile as tile
from concourse import mybir
from concourse import bass_utils

BF16 = mybir.dt.bfloat16
F32 = mybir.dt.float32
F8E4 = mybir.dt.float8e4
NPBF16 = ml_dtypes.bfloat16
NPF8 = ml_dtypes.float8_e4m3fn

B, T, S, E, KV, H, D = 8, 256, 4096, 512, 768, 8, 64
NC_CORES = 8


def _groups(n_sc):
    """Split n_sc 128-wide s-chunks into groups of <=4 (512 ctx cols).

    The remainder group goes FIRST: a tiny group 0 lets the PE start on
    kv-proj as soon as possible (small first ctx DMA), and a full-size
    last group gives the software pipeline real PE work to overlap the
    tail normalization latency with.
    """
    rem = n_sc % 4
    out = []
    sc0 = 0
    if rem:
        out.append((0, rem))
        sc0 = rem
    while sc0 < n_sc:
        out.append((sc0, 4))
        sc0 += 4
    return out


def _build_program(s_pad):
    n_sc = s_pad // 128
    groups = _groups(n_sc)
    n_g = len(groups)

    nc = bacc.Bacc("TRN2", target_bir_lowering=False, debug=False)

    # Host pre-packs every multi-chunk tensor c-side-by-side on 128
    # partitions so each input needs only one or two big DMA transfers —
    # queue postings serialize on completion semaphores, so many small
    # transfers cost ~1.5-2us each in arrival latency.
    ctxp_d = [
        nc.dram_tensor(
            f"ctxp{g}", [128, 6 * 128 * groups[g][1]], BF16, kind="ExternalInput"
        ).ap()
        for g in range(n_g)
    ]
    xp_d = nc.dram_tensor("xp", [128, 4 * T], BF16, kind="ExternalInput").ap()
    m01_d = nc.dram_tensor("m01", [128, n_sc], F32, kind="ExternalInput").ap()
    wqp_d = nc.dram_tensor("wqp", [128, 4 * 512], BF16, kind="ExternalInput").ap()
    wkvp_d = nc.dram_tensor("wkvp", [128, 6 * 512], BF16, kind="ExternalInput").ap()
    wk8p_d = nc.dram_tensor("wk8p", [128, 3 * 1024], F8E4, kind="ExternalInput").ap()
    ctx8p_d = [
        nc.dram_tensor(
            f"ctx8p{g}", [128, 6 * 128 * groups[g][1]], F8E4, kind="ExternalInput"
        ).ap()
        for g in range(n_g)
    ]
    wop_d = nc.dram_tensor("wop", [128, 4 * 512], BF16, kind="ExternalInput").ap()
    bo_d = nc.dram_tensor("bo_r", [128, 4], F32, kind="ExternalInput").ap()
    outT_d = nc.dram_tensor("outT", [4, 128, T], F32, kind="ExternalOutput").ap()

    with tile.TileContext(nc) as tc, ExitStack() as ctx:
        const = ctx.enter_context(tc.tile_pool(name="const", bufs=1))
        work = ctx.enter_context(tc.tile_pool(name="work", bufs=2))
        # PSUM: 3 scores banks + 3 kv-aux banks + 2 PV banks = 8.
        p_sc = ctx.enter_context(tc.tile_pool(name="p_sc", bufs=3, space="PSUM"))
        p_a = ctx.enter_context(tc.tile_pool(name="p_a", bufs=3, space="PSUM"))
        p_pv = ctx.enter_context(tc.tile_pool(name="p_pv", bufs=2, space="PSUM"))

        # ---- static SBUF tensors -------------------------------------------
        ctxp_t = [
            const.tile(
                [128, 6 * 128 * groups[g][1]], BF16, tag=f"ctxp{g}", name=f"ctxp{g}"
            )
            for g in range(n_g)
        ]

        def ctx_slice(g, c, lo, hi):
            w = 128 * groups[g][1]
            return ctxp_t[g][:, c * w + lo : c * w + hi]

        kt_t = [
            const.tile([128, s_pad], BF16, tag=f"kt{kc}", name=f"kt{kc}")
            for kc in range(4)
        ]
        vp_t = [
            const.tile([128, 8 * 65], BF16, tag=f"vp{sc}", name=f"vp{sc}")
            for sc in range(n_sc)
        ]
        qt_t = [
            const.tile([128, T], BF16, tag=f"qt{qc}", name=f"qt{qc}") for qc in range(4)
        ]
        ot_t = [
            const.tile([128, T], BF16, tag=f"ot{cc}", name=f"ot{cc}") for cc in range(4)
        ]
        wqp_t = const.tile([128, 4 * 512], BF16, tag="wqp")
        wkvp_t = const.tile([128, 6 * 512], BF16, tag="wkvp")
        wk8p_t = const.tile([128, 3 * 1024], F8E4, tag="wk8p")
        ctx8p_t = [
            const.tile(
                [128, 6 * 128 * groups[g][1]], F8E4, tag=f"ctx8p{g}", name=f"ctx8p{g}"
            )
            for g in range(n_g)
        ]
        wop_t = const.tile([128, 4 * 512], BF16, tag="wop")
        xp_t = const.tile([128, 4 * T], BF16, tag="xp")

        def wkvv_slice(c):
            return wkvp_t[:, c * 512 : (c + 1) * 512]

        def wk8_slice(r, kc):
            v = wk8p_t[:, r * 1024 : (r + 1) * 1024].rearrange(
                "p (i m) -> p i m", i=2
            )
            return v[:, :, kc * 128 : (kc + 1) * 128]

        def ctx8_slice(g, r):
            w = 128 * groups[g][1]
            return ctx8p_t[g][:, r * 2 * w : (r + 1) * 2 * w].rearrange(
                "p (i s) -> p i s", i=2
            )

        pvacc_t = [
            const.tile([65, T], F32, tag=f"pvacc{h}", name=f"pvacc{h}")
            for h in range(8)
        ]
        den2_t = [
            const.tile([2, T], F32, tag=f"den2_{kc}", name=f"den2_{kc}")
            for kc in range(4)
        ]
        rec2_t = [
            const.tile([2, T], F32, tag=f"rec2_{kc}", name=f"rec2_{kc}")
            for kc in range(4)
        ]
        rech_t = const.tile([1, 8 * T], F32, tag="rech")
        m01_t = const.tile([128, n_sc], F32, tag="m01")
        bo_t = const.tile([128, 4], F32, tag="bo")
        ones8_t = const.tile([128, 8], BF16, tag="ones8")
        ones64_t = const.tile([1, 64], F32, tag="ones64")

        # ---- loads ----------------------------------------------------------
        # 3 DMA queues (sync/SP, gpsimd, scalar/Activation).
        #   sync:   x, wq (Q-proj deps) then later ctx groups
        #   gpsimd: ctx group 0 (tiny) + group 1 then later groups
        #   scalar: wkv (consumed c-outer by group-0 K-part), then cold path
        nc.vector.memset(ones8_t[:], 1.0)
        nc.vector.memset(ones64_t[:], 1.0)
        g0w = 128 * groups[0][1]
        nc.sync.dma_start(xp_t[:], xp_d)
        nc.gpsimd.dma_start(wk8p_t[:], wk8p_d)
        nc.gpsimd.dma_start(ctx8p_t[0][:], ctx8p_d[0])
        nc.sync.dma_start(wqp_t[:], wqp_d)
        nc.scalar.dma_start(wkvp_t[:], wkvp_d)
        nc.gpsimd.dma_start(ctxp_t[0][:], ctxp_d[0])
        if n_g > 1:
            nc.gpsimd.dma_start(ctx8p_t[1][:], ctx8p_d[1])
            nc.sync.dma_start(ctxp_t[1][:], ctxp_d[1])
        nc.scalar.dma_start(m01_t[:], m01_d)
        nc.scalar.dma_start(bo_t[:], bo_d)
        for g in range(2, n_g):
            q = [nc.gpsimd, nc.sync, nc.scalar][g % 3]
            q.dma_start(ctx8p_t[g][:], ctx8p_d[g])
            q.dma_start(ctxp_t[g][:], ctxp_d[g])
        nc.scalar.dma_start(wop_t[:], wop_d)

        # ---- HAM warm-up ----------------------------------------------------
        # The PE clock sits at 1.2 GHz until ~3.4us of sustained activity.
        # Real work is DMA-gated for the first ~6us after the preamble, so
        # burn that window on dependency-free dummy matmuls: by the time x/wq
        # land, the PE runs at 2.4 GHz and stays there.
        warm_w = const.tile([128, 128], BF16, tag="warm_w")
        nc.vector.memset(warm_w[:], 0.0)
        wps = p_a.tile([128, 512], F32, tag="a", name="warm_ps")
        for i in range(56):
            nc.tensor.matmul(
                wps[:, 0:128], lhsT=warm_w[:], rhs=warm_w[:], start=True, stop=True
            )

        # ---- Q projection (PE warm-up while ctx/wkv stream) -----------------
        for qc in range(4):
            qps = p_sc.tile([128, 512], F32, tag="sc", name=f"qps{qc}")
            for ec in range(4):
                nc.tensor.matmul(
                    qps[:, 0:T],
                    lhsT=wqp_t[:, ec * 512 + qc * 128 : ec * 512 + (qc + 1) * 128],
                    rhs=xp_t[:, ec * T : (ec + 1) * T],
                    start=(ec == 0),
                    stop=(ec == 3),
                )
            nc.vector.tensor_copy(qt_t[qc][:], qps[:, 0:T])

        # ---- group-0 K-part (fp8 DoubleRow: kv contracted 256/pass) ---------
        for kc in range(4):
            aps0 = p_sc.tile([128, 512], F32, tag="sc", name=f"aps0_{kc}")
            for r in range(3):
                nc.tensor.matmul(
                    aps0[:, 0:g0w],
                    lhsT=wk8_slice(r, kc),
                    rhs=ctx8_slice(0, r)[:, :, 0:g0w],
                    start=(r == 0),
                    stop=(r == 2),
                    perf_mode=mybir.MatmulPerfMode.DoubleRow,
                )
            nc.vector.tensor_copy(kt_t[kc][:, 0:g0w], aps0[:, 0:g0w])

        sc2g = {}
        for gi, (sc0_, nsc_) in enumerate(groups):
            for sc_ in range(sc0_, sc0_ + nsc_):
                sc2g[sc_] = gi

        def vprime(sc):
            """V' for one 128-wide s-chunk: [128 s, 8h*65] with mask folded."""
            g = sc2g[sc]
            off = (sc - groups[g][0]) * 128
            ps = p_a.tile([128, 512], F32, tag="a", name=f"vps{sc}")
            for c in range(6):
                nc.tensor.matmul(
                    ps[:],
                    lhsT=ctx_slice(g, c, off, off + 128),
                    rhs=wkvv_slice(c),
                    start=(c == 0),
                    stop=(c == 5),
                )
            dst = vp_t[sc][:].rearrange("p (h e) -> p h e", e=65)
            nc.vector.tensor_scalar_mul(
                dst[:, :, 0:64],
                ps[:].rearrange("p (h d) -> p h d", d=64),
                m01_t[:, sc : sc + 1],
            )
            nc.vector.tensor_scalar_mul(
                dst[:, :, 64:65],
                ones8_t[:].rearrange("p (h o) -> p h o", o=1),
                m01_t[:, sc : sc + 1],
            )

        kps_live = {}

        def kpart(g, kc, rs):
            """K-projection slice kc for ctx group g (fp8 DoubleRow passes rs)."""
            sc0, nsc = groups[g]
            w = nsc * 128
            if rs[0] == 0:
                kps_live[0] = p_a.tile([128, 512], F32, tag="a", name=f"kps{g}_{kc}")
            ps = kps_live[0]
            for r in rs:
                nc.tensor.matmul(
                    ps[:, 0:w],
                    lhsT=wk8_slice(r, kc),
                    rhs=ctx8_slice(g, r)[:, :, 0:w],
                    start=(r == 0),
                    stop=(r == 2),
                    perf_mode=mybir.MatmulPerfMode.DoubleRow,
                )
            if rs[-1] == 2:
                nc.vector.tensor_copy(
                    kt_t[kc][:, sc0 * 128 : sc0 * 128 + w], ps[:, 0:w]
                )

        def alloc_exp(g, kc):
            e0 = work.tile([128, 1024], BF16, tag="exp", bufs=8, name=f"e0_{g}_{kc}")
            e1 = work.tile([128, 1024], BF16, tag="exp", bufs=8, name=f"e1_{g}_{kc}")
            exps[(g, kc)] = (e0, e1)

        def scores_half(g, kc, jh):
            """ScoresT + exp for head pair kc, group g, j-half jh (0/1).

            Each half is a [128,<=512] psum pair from the 3-deep rotation;
            one ACT Exp per tile keeps ACT inside the PE's per-half budget.
            """
            sc0, nsc = groups[g]
            j0 = jh * 2
            j1 = min(j0 + 2, nsc)
            if j1 <= j0:
                return
            w = (j1 - j0) * 256
            e0, e1 = exps[(g, kc)]
            pe0 = p_sc.tile([128, 512], F32, tag="sc", name=f"pe0_{g}_{kc}_{jh}")
            pe1 = p_sc.tile([128, 512], F32, tag="sc", name=f"pe1_{g}_{kc}_{jh}")
            for j in range(j0, j1):
                sc = sc0 + j
                nc.tensor.matmul(
                    pe0[:, (j - j0) * 256 : (j - j0 + 1) * 256],
                    lhsT=kt_t[kc][0:64, sc * 128 : (sc + 1) * 128],
                    rhs=qt_t[kc][0:64, :],
                    start=True,
                    stop=True,
                )
                nc.tensor.matmul(
                    pe1[:, (j - j0) * 256 : (j - j0 + 1) * 256],
                    lhsT=kt_t[kc][64:128, sc * 128 : (sc + 1) * 128],
                    rhs=qt_t[kc][64:128, :],
                    start=True,
                    stop=True,
                )
            nc.scalar.activation(
                e0[:, j0 * 256 : j0 * 256 + w], pe0[:, 0:w],
                mybir.ActivationFunctionType.Exp,
            )
            nc.scalar.activation(
                e1[:, j0 * 256 : j0 * 256 + w], pe1[:, 0:w],
                mybir.ActivationFunctionType.Exp,
            )

        def pv(g, kc):
            """PV for head pair kc over group g, accumulate into pvacc."""
            sc0, nsc = groups[g]
            e0, e1 = exps.pop((g, kc))
            pvq0 = p_pv.tile([65, T], F32, tag="pv", name=f"pvq0_{g}_{kc}")
            pvq1 = p_pv.tile([65, T], F32, tag="pv", name=f"pvq1_{g}_{kc}")
            for j in range(nsc):
                sc = sc0 + j
                nc.tensor.matmul(
                    pvq0[:],
                    lhsT=vp_t[sc][:, (2 * kc) * 65 : (2 * kc) * 65 + 65],
                    rhs=e0[:, j * 256 : (j + 1) * 256],
                    start=(j == 0),
                    stop=(j == nsc - 1),
                )
                nc.tensor.matmul(
                    pvq1[:],
                    lhsT=vp_t[sc][:, (2 * kc + 1) * 65 : (2 * kc + 1) * 65 + 65],
                    rhs=e1[:, j * 256 : (j + 1) * 256],
                    start=(j == 0),
                    stop=(j == nsc - 1),
                )
            if g == 0:
                nc.vector.tensor_copy(pvacc_t[2 * kc][:], pvq0[:])
                nc.vector.tensor_copy(pvacc_t[2 * kc + 1][:], pvq1[:])
            else:
                nc.vector.tensor_add(pvacc_t[2 * kc][:], pvacc_t[2 * kc][:], pvq0[:])
                nc.vector.tensor_add(
                    pvacc_t[2 * kc + 1][:], pvacc_t[2 * kc + 1][:], pvq1[:]
                )

        exps = {}

        def den_chain(kc):
            """Collect denominators of head pair kc, reciprocal, refold to rech."""
            nc.sync.dma_start(den2_t[kc][0:1, :], pvacc_t[2 * kc][64:65, :])
            nc.gpsimd.dma_start(den2_t[kc][1:2, :], pvacc_t[2 * kc + 1][64:65, :])
            nc.vector.reciprocal_approx_fast(rec2_t[kc][:], den2_t[kc][:])
            nc.sync.dma_start(
                rech_t[0:1, (2 * kc) * T : (2 * kc + 2) * T].rearrange(
                    "p (h t) -> p h t", t=T
                ),
                rec2_t[kc][:],
            )

        def bc_mul(kc):
            """Broadcast 1/den over 64 partitions (K=1 matmul), scale PV -> OT."""
            bc = p_a.tile([128, 512], F32, tag="a", name=f"bc{kc}")
            nc.tensor.matmul(
                bc[0:64, 0:512],
                lhsT=ones64_t[:],
                rhs=rech_t[0:1, (2 * kc) * T : (2 * kc + 2) * T],
                start=True,
                stop=True,
            )
            tmp1 = work.tile([64, T], BF16, tag="otmp", bufs=2, name=f"otmp{kc}")
            nc.vector.tensor_mul(
                tmp1[:], pvacc_t[2 * kc + 1][0:64, :], bc[0:64, T : 2 * T]
            )
            nc.gpsimd.dma_start(ot_t[kc][64:128, :], tmp1[:])
            nc.vector.tensor_mul(
                ot_t[kc][0:64, :], pvacc_t[2 * kc][0:64, :], bc[0:64, 0:T]
            )

        def outproj_all():
            """Out-proj eo-outer: one [128,512] accumulator at a time from
            the 3-deep rotation; bias + output DMA fire per eo."""
            for eo in range(4):
                reg = p_sc.tile([128, 512], F32, tag="sc", name=f"ops{eo}")
                for kc in range(4):
                    nc.tensor.matmul(
                        reg[:, 0:T],
                        lhsT=wop_t[:, kc * 512 + eo * 128 : kc * 512 + (eo + 1) * 128],
                        rhs=ot_t[kc][:],
                        start=(kc == 0),
                        stop=(kc == 3),
                    )
                osb = work.tile([128, T], F32, tag="osb", bufs=4, name=f"osb{eo}")
                nc.vector.tensor_scalar_add(osb[:], reg[:, 0:T], bo_t[:, eo : eo + 1])
                q = nc.sync if eo % 2 == 0 else nc.gpsimd
                q.dma_start(outT_d[eo], osb[:])

        # ---- V'(0) then pipelined groups -----------------------------------
        for sc in range(groups[0][0], groups[0][0] + groups[0][1]):
            vprime(sc)

        # Single-phase slots: scores/kpart/V'/PV interleaved per head pair
        # (PV shifted one slot so its exp is ready) keeps every engine fed
        # at its own rate instead of alternating ACT-paced and ACT-idle
        # phases.
        for g in range(1, n_g):
            for kc in range(4):
                alloc_exp(g - 1, kc)
                scores_half(g - 1, kc, 0)
                kpart(g, kc, [0, 1])
                scores_half(g - 1, kc, 1)
                kpart(g, kc, [2])
                if kc < groups[g][1]:
                    vprime(groups[g][0] + kc)
                if kc >= 1:
                    pv(g - 1, kc - 1)
            pv(g - 1, 3)

        # ---- final group: scores/PV interleaved with the normalization tail.
        gl = n_g - 1
        for kc in range(4):
            alloc_exp(gl, kc)
            scores_half(gl, kc, 0)
            if kc >= 1:
                pv(gl, kc - 1)
                den_chain(kc - 1)
            scores_half(gl, kc, 1)
            if kc >= 2:
                bc_mul(kc - 2)
        pv(gl, 3)
        den_chain(3)
        bc_mul(2)
        bc_mul(3)
        outproj_all()

    nc.compile()
    return nc


_NC_CACHE = {}


def _get_nc(s_pad):
    if s_pad not in _NC_CACHE:
        _NC_CACHE[s_pad] = _build_program(s_pad)
    return _NC_CACHE[s_pad]


def _pack6(mT):
    """[k*128, N] -> [128, k*N] with chunk c at cols c*N."""
    n = mT.shape[1]
    return np.ascontiguousarray(
        mT.reshape(-1, 128, n).transpose(1, 0, 2).reshape(128, -1)
    )


def _prep_in_maps(x, context, key_padding_mask, Wq, Wkv, Wo, bo):
    keep = [np.flatnonzero(~key_padding_mask[b]) for b in range(B)]
    max_keep = max(len(k) for k in keep)
    s_pad = max(128, -(-max_keep // 128) * 128)
    groups = _groups(s_pad // 128)

    wqp = _pack6((np.ascontiguousarray(Wq.T) * np.float32(D**-0.5)).astype(NPBF16))
    wkvT = np.ascontiguousarray(Wkv.T)  # [768, 1024]
    wkvp = _pack6(np.ascontiguousarray(wkvT[:, 512:1024]).astype(NPBF16))
    wk8p = np.ascontiguousarray(
        wkvT[:, 0:512].reshape(3, 2, 128, 512).transpose(2, 0, 1, 3).reshape(128, -1)
    ).astype(NPF8)
    wop = _pack6(np.ascontiguousarray(Wo.T).astype(NPBF16))
    bo_r = np.ascontiguousarray(bo.reshape(4, 128).T).astype(np.float32)
    in_maps = []
    for b in range(B):
        nk = len(keep[b])
        ctxc = np.zeros((s_pad, KV), dtype=np.float32)
        ctxc[:nk] = context[b][keep[b]]
        ctxT = np.ascontiguousarray(ctxc.T).astype(NPBF16)  # [768, s_pad]
        ctx3 = ctxT.reshape(6, 128, s_pad)
        m = np.zeros(s_pad, dtype=np.float32)
        m[:nk] = 1.0
        m01 = np.ascontiguousarray(m.reshape(s_pad // 128, 128).T)
        ctx8 = np.ascontiguousarray(
            ctxT.astype(np.float32)
            .reshape(3, 2, 128, s_pad)
            .transpose(2, 0, 1, 3)
        ).astype(NPF8)  # [128, 3, 2, s_pad]
        im = dict(
            xp=_pack6(np.ascontiguousarray(x[b].T).astype(NPBF16)),
            m01=m01, wqp=wqp, wkvp=wkvp, wk8p=wk8p, wop=wop, bo_r=bo_r,
        )
        for g, (sc0, nsc) in enumerate(groups):
            blk = ctx3[:, :, sc0 * 128 : (sc0 + nsc) * 128]  # [6,128,w]
            im[f"ctxp{g}"] = np.ascontiguousarray(
                blk.transpose(1, 0, 2).reshape(128, -1)
            )
            im[f"ctx8p{g}"] = np.ascontiguousarray(
                ctx8[:, :, :, sc0 * 128 : (sc0 + nsc) * 128].reshape(128, -1)
            )
        in_maps.append(im)
    return in_maps, s_pad


def _run(inputs, trace=False, **kw):
    in_maps, s_pad = _prep_in_maps(**inputs)
    nc = _get_nc(s_pad)
    res = bass_utils.run_bass_kernel_spmd(
        nc, in_maps, core_ids=list(range(NC_CORES)), trace=trace, **kw
    )
    out = np.stack(
        [res.results[b]["outT"].reshape(E, T).T for b in range(B)]
    ).astype(np.float32)
    return out, res


def kernel(**inputs):
    out, _ = _run(inputs, trace=False)
    return out


if __name__ == "__main__":
    rng = np.random.default_rng(0)
    ins = dict(
        x=rng.standard_normal((B, T, E), dtype=np.float32),
        context=rng.standard_normal((B, S, KV), dtype=np.float32),
        key_padding_mask=rng.integers(0, 2, (B, S)).astype(bool),
        Wq=(rng.standard_normal((512, E), dtype=np.float32) * 0.02),
        Wkv=(rng.standard_normal((1024, KV), dtype=np.float32) * 0.02),
        Wo=(rng.standard_normal((E, 512), dtype=np.float32) * 0.02),
        bo=np.zeros(E, dtype=np.float32),
    )
    out = kernel(**ins)
    print("out", out.shape, out.dtype, np.abs(out).mean())


# revision 46
# speedup vs baseline: 1.1470x; 1.0043x over previous
"""CrossAttention Trainium2 kernel (mask-compacted, fp8 K-projection).

Problem (hardcoded): B=8, T=256, S=4096, E=512, KV=768, H=8, D=64.
Sharding: data-parallel over B — one batch per NeuronCore (8 cores).

Key ideas:
  * ~50% of keys are masked (key_padding_mask True = ignore) and masked
    keys provably contribute nothing (softmax weight exactly 0 via the
    m01 fold into V'), so the host compacts each batch's context to the
    kept keys padded to a common S_pad (multiple of 128; ~2176 for the
    harness seed). All S-proportional device work (KV-proj, scores, exp,
    PV) drops ~1.9x. Padding rows: ctx=0 => k=0 => exp(score)=1, but
    m01=0 zeroes their V' rows and ones-col so they add 0 to numerator
    and denominator. Exact math — compaction changes nothing.
  * K-projection runs in fp8-e4m3 with perf_mode=DoubleRow (2 fp8 MACs
    per PE cell per cycle, kv contracted 256/pass => 3 passes instead of
    6). Only the K path is quantized: scores pass through exp and a
    ~2000-key average, so the ~5% per-element fp8 noise lands at ~1.6e-2
    rel_l2 on the output (vs 4.2e-3 all-bf16, gate 2e-2, deterministic).
    V stays bf16 (V noise passes straight through to the output).
  * Host packs every multi-chunk input side-by-side on 128 partitions so
    each tensor ships in 1-2 large DMA transfers (queue postings
    serialize on completion semaphores; many small transfers cost
    ~1.5-2us each in arrival latency).
  * A dependency-free dummy-matmul warmup burns the DMA-gated start
    window so the PE HAM clock is at 2.4 GHz when real work lands.

Per-core dataflow (bf16 unless noted):
    QT    = wqT.T @ xT -> [512c, 256t]          (scale folded into wq)
    KT    = Wk.T @ ctxT (fp8 DoubleRow) -> [512c, S_pad] (4 head pairs)
    V'    = ctxT.T @ Wv -> per-sc [128 s, 8h*65] * m01 (65th col = m01)
    scoresT[s,t] per head: KT head slices as lhsT (K=64, head pair packed
            into PE row groups 0:64/64:128 -> concurrent row-tiled MMs)
    expsT = Exp(scoresT) on ACT; PV = V'_h @ expsT -> [65,256] psum, row
            64 = softmax denominator; per-group PSUM accumulation then
            DVE-added into SBUF pvacc.
    norm  = reciprocal_approx_fast(denoms), broadcast over 64 partitions
            via a K=1 matmul; OT = PV * recip; outT = woT.T @ OT + bo.

Schedule: single-phase software pipeline over 512-col ctx groups g
(remainder group first). Each slot kc of iteration g interleaves
scores(g-1,kc) halves / kpart(g,kc) fp8 passes / V'(g,kc) / PV(g-1,kc-1)
so PE, ACT (exp), DVE (evictions, V' scaling, PV accumulation) and the
DMA queues all stay fed at their own rates; scores psum rotates through
3 single-bank buffers (PSUM budget: 3 scores + 3 kv-aux + 2 PV = 8
banks). The final group interleaves the per-head-pair denominator
chains (psum row -> reciprocal -> rech), the K=1 broadcast matmuls and
OT scaling into the last scores/PV slots, then the out-projection
accumulates per-eo with bias + output DMA fired as each chunk closes.
"""

import sys

sys.path.insert(0, "/opt/trn_rl_repo")

import numpy as np
import ml_dtypes
from contextlib import ExitStack

import concourse.bass as bass
import concourse.bacc as bacc
import concourse.tile as tile
from concourse import mybir
from concourse import bass_utils

BF16 = mybir.dt.bfloat16
F32 = mybir.dt.float32
F8E4 = mybir.dt.float8e4
NPBF16 = ml_dtypes.bfloat16
NPF8 = ml_dtypes.float8_e4m3fn

B, T, S, E, KV, H, D = 8, 256, 4096, 512, 768, 8, 64
NC_CORES = 8


def _groups(n_sc):
    """Split n_sc 128-wide s-chunks into groups of <=4 (512 ctx cols).

    The remainder group goes FIRST: a tiny group 0 lets the PE start on
    kv-proj as soon as possible (small first ctx DMA), and a full-size
    last group gives the software pipeline real PE work to overlap the
    tail normalization latency with.
    """
    rem = n_sc % 4
    out = []
    sc0 = 0
    if rem:
        out.append((0, rem))
        sc0 = rem
    while sc0 < n_sc:
        out.append((sc0, 4))
        sc0 += 4
    # Split the last full group in two: the tail (norm chains + out-proj)
    # tends to run at the throttled PE clock after micro-gaps, so halving
    # the final group's size halves the work exposed to it while keeping
    # enough PE cover for the denominator-chain latency.
    if len(out) >= 2 and out[-1][1] == 4:
        sc0, _ = out[-1]
        out[-1] = (sc0, 2)
        out.append((sc0 + 2, 2))
    return out


def _build_program(s_pad):
    n_sc = s_pad // 128
    groups = _groups(n_sc)
    n_g = len(groups)

    nc = bacc.Bacc("TRN2", target_bir_lowering=False, debug=False)

    # Host pre-packs every multi-chunk tensor c-side-by-side on 128
    # partitions so each input needs only one or two big DMA transfers —
    # queue postings serialize on completion semaphores, so many small
    # transfers cost ~1.5-2us each in arrival latency.
    ctxp_d = [
        nc.dram_tensor(
            f"ctxp{g}", [128, 6 * 128 * groups[g][1]], BF16, kind="ExternalInput"
        ).ap()
        for g in range(n_g)
    ]
    xp_d = nc.dram_tensor("xp", [128, 4 * T], BF16, kind="ExternalInput").ap()
    m01_d = nc.dram_tensor("m01", [128, n_sc], F32, kind="ExternalInput").ap()
    wqp_d = nc.dram_tensor("wqp", [128, 4 * 512], BF16, kind="ExternalInput").ap()
    wkvp_d = nc.dram_tensor("wkvp", [128, 6 * 512], BF16, kind="ExternalInput").ap()
    wk8p_d = nc.dram_tensor("wk8p", [128, 3 * 1024], F8E4, kind="ExternalInput").ap()
    ctx8p_d = [
        nc.dram_tensor(
            f"ctx8p{g}", [128, 6 * 128 * groups[g][1]], F8E4, kind="ExternalInput"
        ).ap()
        for g in range(n_g)
    ]
    wop_d = nc.dram_tensor("wop", [128, 4 * 512], BF16, kind="ExternalInput").ap()
    bo_d = nc.dram_tensor("bo_r", [128, 4], F32, kind="ExternalInput").ap()
    outT_d = nc.dram_tensor("outT", [4, 128, T], F32, kind="ExternalOutput").ap()

    with tile.TileContext(nc) as tc, ExitStack() as ctx:
        const = ctx.enter_context(tc.tile_pool(name="const", bufs=1))
        work = ctx.enter_context(tc.tile_pool(name="work", bufs=2))
        # PSUM: 3 scores banks + 3 kv-aux banks + 2 PV banks = 8.
        p_sc = ctx.enter_context(tc.tile_pool(name="p_sc", bufs=3, space="PSUM"))
        p_a = ctx.enter_context(tc.tile_pool(name="p_a", bufs=3, space="PSUM"))
        p_pv = ctx.enter_context(tc.tile_pool(name="p_pv", bufs=2, space="PSUM"))

        # ---- static SBUF tensors -------------------------------------------
        ctxp_t = [
            const.tile(
                [128, 6 * 128 * groups[g][1]], BF16, tag=f"ctxp{g}", name=f"ctxp{g}"
            )
            for g in range(n_g)
        ]

        def ctx_slice(g, c, lo, hi):
            w = 128 * groups[g][1]
            return ctxp_t[g][:, c * w + lo : c * w + hi]

        kt_t = [
            const.tile([128, s_pad], BF16, tag=f"kt{kc}", name=f"kt{kc}")
            for kc in range(4)
        ]
        vp_t = [
            const.tile([128, 8 * 65], BF16, tag=f"vp{sc}", name=f"vp{sc}")
            for sc in range(n_sc)
        ]
        qt_t = [
            const.tile([128, T], BF16, tag=f"qt{qc}", name=f"qt{qc}") for qc in range(4)
        ]
        ot_t = [
            const.tile([128, T], BF16, tag=f"ot{cc}", name=f"ot{cc}") for cc in range(4)
        ]
        wqp_t = const.tile([128, 4 * 512], BF16, tag="wqp")
        wkvp_t = const.tile([128, 6 * 512], BF16, tag="wkvp")
        wk8p_t = const.tile([128, 3 * 1024], F8E4, tag="wk8p")
        ctx8p_t = [
            const.tile(
                [128, 6 * 128 * groups[g][1]], F8E4, tag=f"ctx8p{g}", name=f"ctx8p{g}"
            )
            for g in range(n_g)
        ]
        wop_t = const.tile([128, 4 * 512], BF16, tag="wop")
        xp_t = const.tile([128, 4 * T], BF16, tag="xp")

        def wkvv_slice(c):
            return wkvp_t[:, c * 512 : (c + 1) * 512]

        def wk8_slice(r, kc):
            v = wk8p_t[:, r * 1024 : (r + 1) * 1024].rearrange(
                "p (i m) -> p i m", i=2
            )
            return v[:, :, kc * 128 : (kc + 1) * 128]

        def ctx8_slice(g, r):
            w = 128 * groups[g][1]
            return ctx8p_t[g][:, r * 2 * w : (r + 1) * 2 * w].rearrange(
                "p (i s) -> p i s", i=2
            )

        pvacc_t = [
            const.tile([65, T], F32, tag=f"pvacc{h}", name=f"pvacc{h}")
            for h in range(8)
        ]
        den2_t = [
            const.tile([2, T], F32, tag=f"den2_{kc}", name=f"den2_{kc}")
            for kc in range(4)
        ]
        rec2_t = [
            const.tile([2, T], F32, tag=f"rec2_{kc}", name=f"rec2_{kc}")
            for kc in range(4)
        ]
        rech_t = const.tile([1, 8 * T], F32, tag="rech")
        m01_t = const.tile([128, n_sc], F32, tag="m01")
        bo_t = const.tile([128, 4], F32, tag="bo")
        ones8_t = const.tile([128, 8], BF16, tag="ones8")
        ones64_t = const.tile([1, 64], F32, tag="ones64")

        # ---- loads ----------------------------------------------------------
        # 3 DMA queues (sync/SP, gpsimd, scalar/Activation).
        #   sync:   x, wq (Q-proj deps) then later ctx groups
        #   gpsimd: ctx group 0 (tiny) + group 1 then later groups
        #   scalar: wkv (consumed c-outer by group-0 K-part), then cold path
        nc.vector.memset(ones8_t[:], 1.0)
        nc.vector.memset(ones64_t[:], 1.0)
        g0w = 128 * groups[0][1]
        nc.sync.dma_start(xp_t[:], xp_d)
        nc.gpsimd.dma_start(wk8p_t[:], wk8p_d)
        nc.gpsimd.dma_start(ctx8p_t[0][:], ctx8p_d[0])
        nc.sync.dma_start(wqp_t[:], wqp_d)
        nc.scalar.dma_start(wkvp_t[:], wkvp_d)
        nc.gpsimd.dma_start(ctxp_t[0][:], ctxp_d[0])
        if n_g > 1:
            nc.gpsimd.dma_start(ctx8p_t[1][:], ctx8p_d[1])
            nc.sync.dma_start(ctxp_t[1][:], ctxp_d[1])
        nc.scalar.dma_start(m01_t[:], m01_d)
        nc.scalar.dma_start(bo_t[:], bo_d)
        for g in range(2, n_g):
            q = [nc.gpsimd, nc.sync, nc.scalar][g % 3]
            q.dma_start(ctx8p_t[g][:], ctx8p_d[g])
            q.dma_start(ctxp_t[g][:], ctxp_d[g])
        nc.scalar.dma_start(wop_t[:], wop_d)

        # ---- HAM warm-up ----------------------------------------------------
        # The PE clock sits at 1.2 GHz until ~3.4us of sustained activity.
        # Real work is DMA-gated for the first ~6us after the preamble, so
        # burn that window on dependency-free dummy matmuls: by the time x/wq
        # land, the PE runs at 2.4 GHz and stays there.
        warm_w = const.tile([128, 128], BF16, tag="warm_w")
        nc.vector.memset(warm_w[:], 0.0)
        wps = p_a.tile([128, 512], F32, tag="a", name="warm_ps")
        for i in range(56):
            nc.tensor.matmul(
                wps[:, 0:128], lhsT=warm_w[:], rhs=warm_w[:], start=True, stop=True
            )

        # ---- Q projection (PE warm-up while ctx/wkv stream) -----------------
        for qc in range(4):
            qps = p_sc.tile([128, 512], F32, tag="sc", name=f"qps{qc}")
            for ec in range(4):
                nc.tensor.matmul(
                    qps[:, 0:T],
                    lhsT=wqp_t[:, ec * 512 + qc * 128 : ec * 512 + (qc + 1) * 128],
                    rhs=xp_t[:, ec * T : (ec + 1) * T],
                    start=(ec == 0),
                    stop=(ec == 3),
                )
            nc.vector.tensor_copy(qt_t[qc][:], qps[:, 0:T])

        # ---- group-0 K-part (fp8 DoubleRow: kv contracted 256/pass) ---------
        for kc in range(4):
            aps0 = p_sc.tile([128, 512], F32, tag="sc", name=f"aps0_{kc}")
            for r in range(3):
                nc.tensor.matmul(
                    aps0[:, 0:g0w],
                    lhsT=wk8_slice(r, kc),
                    rhs=ctx8_slice(0, r)[:, :, 0:g0w],
                    start=(r == 0),
                    stop=(r == 2),
                    perf_mode=mybir.MatmulPerfMode.DoubleRow,
                )
            nc.vector.tensor_copy(kt_t[kc][:, 0:g0w], aps0[:, 0:g0w])

        sc2g = {}
        for gi, (sc0_, nsc_) in enumerate(groups):
            for sc_ in range(sc0_, sc0_ + nsc_):
                sc2g[sc_] = gi

        def vprime(sc):
            """V' for one 128-wide s-chunk: [128 s, 8h*65] with mask folded."""
            g = sc2g[sc]
            off = (sc - groups[g][0]) * 128
            ps = p_a.tile([128, 512], F32, tag="a", name=f"vps{sc}")
            for c in range(6):
                nc.tensor.matmul(
                    ps[:],
                    lhsT=ctx_slice(g, c, off, off + 128),
                    rhs=wkvv_slice(c),
                    start=(c == 0),
                    stop=(c == 5),
                )
            dst = vp_t[sc][:].rearrange("p (h e) -> p h e", e=65)
            nc.vector.tensor_scalar_mul(
                dst[:, :, 0:64],
                ps[:].rearrange("p (h d) -> p h d", d=64),
                m01_t[:, sc : sc + 1],
            )
            nc.vector.tensor_scalar_mul(
                dst[:, :, 64:65],
                ones8_t[:].rearrange("p (h o) -> p h o", o=1),
                m01_t[:, sc : sc + 1],
            )

        kps_live = {}

        def kpart(g, kc, rs):
            """K-projection slice kc for ctx group g (fp8 DoubleRow passes rs)."""
            sc0, nsc = groups[g]
            w = nsc * 128
            if rs[0] == 0:
                kps_live[0] = p_a.tile([128, 512], F32, tag="a", name=f"kps{g}_{kc}")
            ps = kps_live[0]
            for r in rs:
                nc.tensor.matmul(
                    ps[:, 0:w],
                    lhsT=wk8_slice(r, kc),
                    rhs=ctx8_slice(g, r)[:, :, 0:w],
                    start=(r == 0),
                    stop=(r == 2),
                    perf_mode=mybir.MatmulPerfMode.DoubleRow,
                )
            if rs[-1] == 2:
                nc.vector.tensor_copy(
                    kt_t[kc][:, sc0 * 128 : sc0 * 128 + w], ps[:, 0:w]
                )

        def alloc_exp(g, kc):
            e0 = work.tile([128, 1024], BF16, tag="exp", bufs=8, name=f"e0_{g}_{kc}")
            e1 = work.tile([128, 1024], BF16, tag="exp", bufs=8, name=f"e1_{g}_{kc}")
            exps[(g, kc)] = (e0, e1)

        def scores_half(g, kc, jh):
            """ScoresT + exp for head pair kc, group g, j-half jh (0/1).

            Each half is a [128,<=512] psum pair from the 3-deep rotation;
            one ACT Exp per tile keeps ACT inside the PE's per-half budget.
            """
            sc0, nsc = groups[g]
            j0 = jh * 2
            j1 = min(j0 + 2, nsc)
            if j1 <= j0:
                return
            w = (j1 - j0) * 256
            e0, e1 = exps[(g, kc)]
            pe0 = p_sc.tile([128, 512], F32, tag="sc", name=f"pe0_{g}_{kc}_{jh}")
            pe1 = p_sc.tile([128, 512], F32, tag="sc", name=f"pe1_{g}_{kc}_{jh}")
            for j in range(j0, j1):
                sc = sc0 + j
                nc.tensor.matmul(
                    pe0[:, (j - j0) * 256 : (j - j0 + 1) * 256],
                    lhsT=kt_t[kc][0:64, sc * 128 : (sc + 1) * 128],
                    rhs=qt_t[kc][0:64, :],
                    start=True,
                    stop=True,
                )
                nc.tensor.matmul(
                    pe1[:, (j - j0) * 256 : (j - j0 + 1) * 256],
                    lhsT=kt_t[kc][64:128, sc * 128 : (sc + 1) * 128],
                    rhs=qt_t[kc][64:128, :],
                    start=True,
                    stop=True,
                )
            nc.scalar.activation(
                e0[:, j0 * 256 : j0 * 256 + w], pe0[:, 0:w],
                mybir.ActivationFunctionType.Exp,
            )
            nc.scalar.activation(
                e1[:, j0 * 256 : j0 * 256 + w], pe1[:, 0:w],
                mybir.ActivationFunctionType.Exp,
            )

        def pv(g, kc):
            """PV for head pair kc over group g, accumulate into pvacc."""
            sc0, nsc = groups[g]
            e0, e1 = exps.pop((g, kc))
            pvq0 = p_pv.tile([65, T], F32, tag="pv", name=f"pvq0_{g}_{kc}")
            pvq1 = p_pv.tile([65, T], F32, tag="pv", name=f"pvq1_{g}_{kc}")
            for j in range(nsc):
                sc = sc0 + j
                nc.tensor.matmul(
                    pvq0[:],
                    lhsT=vp_t[sc][:, (2 * kc) * 65 : (2 * kc) * 65 + 65],
                    rhs=e0[:, j * 256 : (j + 1) * 256],
                    start=(j == 0),
                    stop=(j == nsc - 1),
                )
                nc.tensor.matmul(
                    pvq1[:],
                    lhsT=vp_t[sc][:, (2 * kc + 1) * 65 : (2 * kc + 1) * 65 + 65],
                    rhs=e1[:, j * 256 : (j + 1) * 256],
                    start=(j == 0),
                    stop=(j == nsc - 1),
                )
            if g == 0:
                nc.vector.tensor_copy(pvacc_t[2 * kc][:], pvq0[:])
                nc.vector.tensor_copy(pvacc_t[2 * kc + 1][:], pvq1[:])
            else:
                nc.vector.tensor_add(pvacc_t[2 * kc][:], pvacc_t[2 * kc][:], pvq0[:])
                nc.vector.tensor_add(
                    pvacc_t[2 * kc + 1][:], pvacc_t[2 * kc + 1][:], pvq1[:]
                )

        exps = {}

        def den_chain(kc):
            """Collect denominators of head pair kc, reciprocal, refold to rech."""
            nc.sync.dma_start(den2_t[kc][0:1, :], pvacc_t[2 * kc][64:65, :])
            nc.gpsimd.dma_start(den2_t[kc][1:2, :], pvacc_t[2 * kc + 1][64:65, :])
            nc.vector.reciprocal_approx_fast(rec2_t[kc][:], den2_t[kc][:])
            nc.sync.dma_start(
                rech_t[0:1, (2 * kc) * T : (2 * kc + 2) * T].rearrange(
                    "p (h t) -> p h t", t=T
                ),
                rec2_t[kc][:],
            )

        def bc_mul(kc):
            """Broadcast 1/den over 64 partitions (K=1 matmul), scale PV -> OT."""
            bc = p_a.tile([128, 512], F32, tag="a", name=f"bc{kc}")
            nc.tensor.matmul(
                bc[0:64, 0:512],
                lhsT=ones64_t[:],
                rhs=rech_t[0:1, (2 * kc) * T : (2 * kc + 2) * T],
                start=True,
                stop=True,
            )
            tmp1 = work.tile([64, T], BF16, tag="otmp", bufs=2, name=f"otmp{kc}")
            nc.vector.tensor_mul(
                tmp1[:], pvacc_t[2 * kc + 1][0:64, :], bc[0:64, T : 2 * T]
            )
            nc.gpsimd.dma_start(ot_t[kc][64:128, :], tmp1[:])
            nc.vector.tensor_mul(
                ot_t[kc][0:64, :], pvacc_t[2 * kc][0:64, :], bc[0:64, 0:T]
            )

        def outproj_all():
            """Out-proj eo-outer: one [128,512] accumulator at a time from
            the 3-deep rotation; bias + output DMA fire per eo."""
            for eo in range(4):
                reg = p_sc.tile([128, 512], F32, tag="sc", name=f"ops{eo}")
                for kc in range(4):
                    nc.tensor.matmul(
                        reg[:, 0:T],
                        lhsT=wop_t[:, kc * 512 + eo * 128 : kc * 512 + (eo + 1) * 128],
                        rhs=ot_t[kc][:],
                        start=(kc == 0),
                        stop=(kc == 3),
                    )
                osb = work.tile([128, T], F32, tag="osb", bufs=4, name=f"osb{eo}")
                nc.vector.tensor_scalar_add(osb[:], reg[:, 0:T], bo_t[:, eo : eo + 1])
                q = nc.sync if eo % 2 == 0 else nc.gpsimd
                q.dma_start(outT_d[eo], osb[:])

        # ---- V'(0) then pipelined groups -----------------------------------
        for sc in range(groups[0][0], groups[0][0] + groups[0][1]):
            vprime(sc)

        # Single-phase slots: scores/kpart/V'/PV interleaved per head pair
        # (PV shifted one slot so its exp is ready) keeps every engine fed
        # at its own rate instead of alternating ACT-paced and ACT-idle
        # phases.
        for g in range(1, n_g):
            for kc in range(4):
                alloc_exp(g - 1, kc)
                scores_half(g - 1, kc, 0)
                kpart(g, kc, [0, 1])
                scores_half(g - 1, kc, 1)
                kpart(g, kc, [2])
                if kc < groups[g][1]:
                    vprime(groups[g][0] + kc)
                if kc >= 1:
                    pv(g - 1, kc - 1)
            pv(g - 1, 3)

        # ---- final group: scores/PV interleaved with the normalization tail.
        gl = n_g - 1
        for kc in range(4):
            alloc_exp(gl, kc)
            scores_half(gl, kc, 0)
            if kc >= 1:
                pv(gl, kc - 1)
                den_chain(kc - 1)
            scores_half(gl, kc, 1)
            if kc >= 2:
                bc_mul(kc - 2)
        pv(gl, 3)
        den_chain(3)
        bc_mul(2)
        bc_mul(3)
        outproj_all()

    nc.compile()
    return nc


_NC_CACHE = {}


def _get_nc(s_pad):
    if s_pad not in _NC_CACHE:
        _NC_CACHE[s_pad] = _build_program(s_pad)
    return _NC_CACHE[s_pad]


def _pack6(mT):
    """[k*128, N] -> [128, k*N] with chunk c at cols c*N."""
    n = mT.shape[1]
    return np.ascontiguousarray(
        mT.reshape(-1, 128, n).transpose(1, 0, 2).reshape(128, -1)
    )


def _prep_in_maps(x, context, key_padding_mask, Wq, Wkv, Wo, bo):
    keep = [np.flatnonzero(~key_padding_mask[b]) for b in range(B)]
    max_keep = max(len(k) for k in keep)
    s_pad = max(128, -(-max_keep // 128) * 128)
    groups = _groups(s_pad // 128)

    wqp = _pack6((np.ascontiguousarray(Wq.T) * np.float32(D**-0.5)).astype(NPBF16))
    wkvT = np.ascontiguousarray(Wkv.T)  # [768, 1024]
    wkvp = _pack6(np.ascontiguousarray(wkvT[:, 512:1024]).astype(NPBF16))
    wk8p = np.ascontiguousarray(
        wkvT[:, 0:512].reshape(3, 2, 128, 512).transpose(2, 0, 1, 3).reshape(128, -1)
    ).astype(NPF8)
    wop = _pack6(np.ascontiguousarray(Wo.T).astype(NPBF16))
    bo_r = np.ascontiguousarray(bo.reshape(4, 128).T).astype(np.float32)
    in_maps = []
    for b in range(B):
        nk = len(keep[b])
        ctxc = np.zeros((s_pad, KV), dtype=np.float32)
        ctxc[:nk] = context[b][keep[b]]
        ctxT = np.ascontiguousarray(ctxc.T).astype(NPBF16)  # [768, s_pad]
        ctx3 = ctxT.reshape(6, 128, s_pad)
        m = np.zeros(s_pad, dtype=np.float32)
        m[:nk] = 1.0
        m01 = np.ascontiguousarray(m.reshape(s_pad // 128, 128).T)
        ctx8 = np.ascontiguousarray(
            ctxT.astype(np.float32)
            .reshape(3, 2, 128, s_pad)
            .transpose(2, 0, 1, 3)
        ).astype(NPF8)  # [128, 3, 2, s_pad]
        im = dict(
            xp=_pack6(np.ascontiguousarray(x[b].T).astype(NPBF16)),
            m01=m01, wqp=wqp, wkvp=wkvp, wk8p=wk8p, wop=wop, bo_r=bo_r,
        )
        for g, (sc0, nsc) in enumerate(groups):
            blk = ctx3[:, :, sc0 * 128 : (sc0 + nsc) * 128]  # [6,128,w]
            im[f"ctxp{g}"] = np.ascontiguousarray(
                blk.transpose(1, 0, 2).reshape(128, -1)
            )
            im[f"ctx8p{g}"] = np.ascontiguousarray(
                ctx8[:, :, :, sc0 * 128 : (sc0 + nsc) * 128].reshape(128, -1)
            )
        in_maps.append(im)
    return in_maps, s_pad


def _run(inputs, trace=False, **kw):
    in_maps, s_pad = _prep_in_maps(**inputs)
    nc = _get_nc(s_pad)
    res = bass_utils.run_bass_kernel_spmd(
        nc, in_maps, core_ids=list(range(NC_CORES)), trace=trace, **kw
    )
    out = np.stack(
        [res.results[b]["outT"].reshape(E, T).T for b in range(B)]
    ).astype(np.float32)
    return out, res


def kernel(**inputs):
    out, _ = _run(inputs, trace=False)
    return out


if __name__ == "__main__":
    rng = np.random.default_rng(0)
    ins = dict(
        x=rng.standard_normal((B, T, E), dtype=np.float32),
        context=rng.standard_normal((B, S, KV), dtype=np.float32),
        key_padding_mask=rng.integers(0, 2, (B, S)).astype(bool),
        Wq=(rng.standard_normal((512, E), dtype=np.float32) * 0.02),
        Wkv=(rng.standard_normal((1024, KV), dtype=np.float32) * 0.02),
        Wo=(rng.standard_normal((E, 512), dtype=np.float32) * 0.02),
        bo=np.zeros(E, dtype=np.float32),
    )
    out = kernel(**ins)
    print("out", out.shape, out.dtype, np.abs(out).mean())


# revision 47
# speedup vs baseline: 1.1576x; 1.0092x over previous
"""CrossAttention Trainium2 kernel (mask-compacted, fp8 K-projection).

Problem (hardcoded): B=8, T=256, S=4096, E=512, KV=768, H=8, D=64.
Sharding: data-parallel over B — one batch per NeuronCore (8 cores).

Key ideas:
  * ~50% of keys are masked (key_padding_mask True = ignore) and masked
    keys provably contribute nothing (softmax weight exactly 0 via the
    m01 fold into V'), so the host compacts each batch's context to the
    kept keys padded to a common S_pad (multiple of 128; ~2176 for the
    harness seed). All S-proportional device work (KV-proj, scores, exp,
    PV) drops ~1.9x. Padding rows: ctx=0 => k=0 => exp(score)=1, but
    m01=0 zeroes their V' rows and ones-col so they add 0 to numerator
    and denominator. Exact math — compaction changes nothing.
  * K-projection runs in fp8-e4m3 with perf_mode=DoubleRow (2 fp8 MACs
    per PE cell per cycle, kv contracted 256/pass => 3 passes instead of
    6). Only the K path is quantized: scores pass through exp and a
    ~2000-key average, so the ~5% per-element fp8 noise lands at ~1.6e-2
    rel_l2 on the output (vs 4.2e-3 all-bf16, gate 2e-2, deterministic).
    V stays bf16 (V noise passes straight through to the output).
  * Host packs every multi-chunk input side-by-side on 128 partitions so
    each tensor ships in 1-2 large DMA transfers (queue postings
    serialize on completion semaphores; many small transfers cost
    ~1.5-2us each in arrival latency).
  * A dependency-free dummy-matmul warmup burns the DMA-gated start
    window so the PE HAM clock is at 2.4 GHz when real work lands.

Per-core dataflow (bf16 unless noted):
    QT    = wqT.T @ xT -> [512c, 256t]          (scale folded into wq)
    KT    = Wk.T @ ctxT (fp8 DoubleRow) -> [512c, S_pad] (4 head pairs)
    V'    = ctxT.T @ Wv -> per-sc [128 s, 8h*65] * m01 (65th col = m01)
    scoresT[s,t] per head: KT head slices as lhsT (K=64, head pair packed
            into PE row groups 0:64/64:128 -> concurrent row-tiled MMs)
    expsT = Exp(scoresT) on ACT; PV = V'_h @ expsT -> [65,256] psum, row
            64 = softmax denominator; per-group PSUM accumulation then
            DVE-added into SBUF pvacc.
    norm  = reciprocal_approx_fast(denoms), broadcast over 64 partitions
            via a K=1 matmul; OT = PV * recip; outT = woT.T @ OT + bo.

Schedule: single-phase software pipeline over 512-col ctx groups g
(remainder group first). Each slot kc of iteration g interleaves
scores(g-1,kc) halves / kpart(g,kc) fp8 passes / V'(g,kc) / PV(g-1,kc-1)
so PE, ACT (exp), DVE (evictions, V' scaling, PV accumulation) and the
DMA queues all stay fed at their own rates; scores psum rotates through
3 single-bank buffers (PSUM budget: 3 scores + 3 kv-aux + 2 PV = 8
banks). The final group interleaves the per-head-pair denominator
chains (psum row -> reciprocal -> rech), the K=1 broadcast matmuls and
OT scaling into the last scores/PV slots, then the out-projection
accumulates per-eo with bias + output DMA fired as each chunk closes.
"""

import sys

sys.path.insert(0, "/opt/trn_rl_repo")

import numpy as np
import ml_dtypes
from contextlib import ExitStack

import concourse.bass as bass
import concourse.bacc as bacc
import concourse.tile as tile
from concourse import mybir
from concourse import bass_utils

BF16 = mybir.dt.bfloat16
F32 = mybir.dt.float32
F8E4 = mybir.dt.float8e4
NPBF16 = ml_dtypes.bfloat16
NPF8 = ml_dtypes.float8_e4m3fn

B, T, S, E, KV, H, D = 8, 256, 4096, 512, 768, 8, 64
NC_CORES = 8


def _groups(n_sc):
    """Split n_sc 128-wide s-chunks into groups of <=4 (512 ctx cols).

    The remainder group goes FIRST: a tiny group 0 lets the PE start on
    kv-proj as soon as possible (small first ctx DMA), and a full-size
    last group gives the software pipeline real PE work to overlap the
    tail normalization latency with.
    """
    rem = n_sc % 4
    out = []
    sc0 = 0
    if rem:
        out.append((0, rem))
        sc0 = rem
    while sc0 < n_sc:
        out.append((sc0, 4))
        sc0 += 4
    return out


def _build_program(s_pad):
    n_sc = s_pad // 128
    groups = _groups(n_sc)
    n_g = len(groups)

    nc = bacc.Bacc("TRN2", target_bir_lowering=False, debug=False)

    # Host pre-packs every multi-chunk tensor c-side-by-side on 128
    # partitions so each input needs only one or two big DMA transfers —
    # queue postings serialize on completion semaphores, so many small
    # transfers cost ~1.5-2us each in arrival latency.
    ctxp_d = [
        nc.dram_tensor(
            f"ctxp{g}", [128, 6 * 128 * groups[g][1]], BF16, kind="ExternalInput"
        ).ap()
        for g in range(n_g)
    ]
    xp_d = nc.dram_tensor("xp", [128, 4 * T], BF16, kind="ExternalInput").ap()
    m01_d = nc.dram_tensor("m01", [128, n_sc], F32, kind="ExternalInput").ap()
    wqp_d = nc.dram_tensor("wqp", [128, 4 * 512], BF16, kind="ExternalInput").ap()
    wkvp_d = nc.dram_tensor("wkvp", [128, 6 * 512], BF16, kind="ExternalInput").ap()
    wk8p_d = nc.dram_tensor("wk8p", [128, 3 * 1024], F8E4, kind="ExternalInput").ap()
    ctx8p_d = [
        nc.dram_tensor(
            f"ctx8p{g}", [128, 6 * 128 * groups[g][1]], F8E4, kind="ExternalInput"
        ).ap()
        for g in range(n_g)
    ]
    wop_d = nc.dram_tensor("wop", [128, 4 * 512], BF16, kind="ExternalInput").ap()
    bo_d = nc.dram_tensor("bo_r", [128, 4], F32, kind="ExternalInput").ap()
    outT_d = nc.dram_tensor("outT", [4, 128, T], F32, kind="ExternalOutput").ap()

    with tile.TileContext(nc) as tc, ExitStack() as ctx:
        const = ctx.enter_context(tc.tile_pool(name="const", bufs=1))
        work = ctx.enter_context(tc.tile_pool(name="work", bufs=2))
        # PSUM: 3 scores banks + 3 kv-aux banks + 2 PV banks = 8.
        p_sc = ctx.enter_context(tc.tile_pool(name="p_sc", bufs=3, space="PSUM"))
        p_a = ctx.enter_context(tc.tile_pool(name="p_a", bufs=3, space="PSUM"))
        p_pv = ctx.enter_context(tc.tile_pool(name="p_pv", bufs=2, space="PSUM"))

        # ---- static SBUF tensors -------------------------------------------
        ctxp_t = [
            const.tile(
                [128, 6 * 128 * groups[g][1]], BF16, tag=f"ctxp{g}", name=f"ctxp{g}"
            )
            for g in range(n_g)
        ]

        def ctx_slice(g, c, lo, hi):
            w = 128 * groups[g][1]
            return ctxp_t[g][:, c * w + lo : c * w + hi]

        kt_t = [
            const.tile([128, s_pad], BF16, tag=f"kt{kc}", name=f"kt{kc}")
            for kc in range(4)
        ]
        vp_t = [
            const.tile([128, 8 * 65], BF16, tag=f"vp{sc}", name=f"vp{sc}")
            for sc in range(n_sc)
        ]
        qt_t = [
            const.tile([128, T], BF16, tag=f"qt{qc}", name=f"qt{qc}") for qc in range(4)
        ]
        ot_t = [
            const.tile([128, T], BF16, tag=f"ot{cc}", name=f"ot{cc}") for cc in range(4)
        ]
        wqp_t = const.tile([128, 4 * 512], BF16, tag="wqp")
        wkvp_t = const.tile([128, 6 * 512], BF16, tag="wkvp")
        wk8p_t = const.tile([128, 3 * 1024], F8E4, tag="wk8p")
        ctx8p_t = [
            const.tile(
                [128, 6 * 128 * groups[g][1]], F8E4, tag=f"ctx8p{g}", name=f"ctx8p{g}"
            )
            for g in range(n_g)
        ]
        wop_t = const.tile([128, 4 * 512], BF16, tag="wop")
        xp_t = const.tile([128, 4 * T], BF16, tag="xp")

        def wkvv_slice(c):
            return wkvp_t[:, c * 512 : (c + 1) * 512]

        def wk8_slice(r, kc):
            v = wk8p_t[:, r * 1024 : (r + 1) * 1024].rearrange(
                "p (i m) -> p i m", i=2
            )
            return v[:, :, kc * 128 : (kc + 1) * 128]

        def ctx8_slice(g, r):
            w = 128 * groups[g][1]
            return ctx8p_t[g][:, r * 2 * w : (r + 1) * 2 * w].rearrange(
                "p (i s) -> p i s", i=2
            )

        pvacc_t = [
            const.tile([65, T], F32, tag=f"pvacc{h}", name=f"pvacc{h}")
            for h in range(8)
        ]
        den2_t = [
            const.tile([2, T], F32, tag=f"den2_{kc}", name=f"den2_{kc}")
            for kc in range(4)
        ]
        rec2_t = [
            const.tile([2, T], F32, tag=f"rec2_{kc}", name=f"rec2_{kc}")
            for kc in range(4)
        ]
        rech_t = const.tile([1, 8 * T], F32, tag="rech")
        m01_t = const.tile([128, n_sc], F32, tag="m01")
        bo_t = const.tile([128, 4], F32, tag="bo")
        ones8_t = const.tile([128, 8], BF16, tag="ones8")
        ones64_t = const.tile([1, 64], F32, tag="ones64")

        # ---- loads ----------------------------------------------------------
        # 3 DMA queues (sync/SP, gpsimd, scalar/Activation).
        #   sync:   x, wq (Q-proj deps) then later ctx groups
        #   gpsimd: ctx group 0 (tiny) + group 1 then later groups
        #   scalar: wkv (consumed c-outer by group-0 K-part), then cold path
        nc.vector.memset(ones8_t[:], 1.0)
        nc.vector.memset(ones64_t[:], 1.0)
        g0w = 128 * groups[0][1]
        nc.sync.dma_start(xp_t[:], xp_d)
        nc.gpsimd.dma_start(wk8p_t[:], wk8p_d)
        nc.gpsimd.dma_start(ctx8p_t[0][:], ctx8p_d[0])
        nc.sync.dma_start(wqp_t[:], wqp_d)
        nc.scalar.dma_start(wkvp_t[:], wkvp_d)
        nc.gpsimd.dma_start(ctxp_t[0][:], ctxp_d[0])
        if n_g > 1:
            nc.gpsimd.dma_start(ctx8p_t[1][:], ctx8p_d[1])
            nc.sync.dma_start(ctxp_t[1][:], ctxp_d[1])
        nc.scalar.dma_start(m01_t[:], m01_d)
        nc.scalar.dma_start(bo_t[:], bo_d)
        for g in range(2, n_g):
            q = [nc.gpsimd, nc.sync, nc.scalar][g % 3]
            q.dma_start(ctx8p_t[g][:], ctx8p_d[g])
            q.dma_start(ctxp_t[g][:], ctxp_d[g])
        nc.scalar.dma_start(wop_t[:], wop_d)

        # ---- HAM warm-up ----------------------------------------------------
        # The PE clock sits at 1.2 GHz until ~3.4us of sustained activity.
        # Real work is DMA-gated for the first ~6us after the preamble, so
        # burn that window on dependency-free dummy matmuls: by the time x/wq
        # land, the PE runs at 2.4 GHz and stays there.
        warm_w = const.tile([128, 128], BF16, tag="warm_w")
        nc.vector.memset(warm_w[:], 0.0)
        wps = p_a.tile([128, 512], F32, tag="a", name="warm_ps")
        for i in range(56):
            nc.tensor.matmul(
                wps[:, 0:128], lhsT=warm_w[:], rhs=warm_w[:], start=True, stop=True
            )

        # ---- Q projection (PE warm-up while ctx/wkv stream) -----------------
        for qc in range(4):
            qps = p_sc.tile([128, 512], F32, tag="sc", name=f"qps{qc}")
            for ec in range(4):
                nc.tensor.matmul(
                    qps[:, 0:T],
                    lhsT=wqp_t[:, ec * 512 + qc * 128 : ec * 512 + (qc + 1) * 128],
                    rhs=xp_t[:, ec * T : (ec + 1) * T],
                    start=(ec == 0),
                    stop=(ec == 3),
                )
            nc.vector.tensor_copy(qt_t[qc][:], qps[:, 0:T])

        # ---- group-0 K-part (fp8 DoubleRow: kv contracted 256/pass) ---------
        for kc in range(4):
            aps0 = p_sc.tile([128, 512], F32, tag="sc", name=f"aps0_{kc}")
            for r in range(3):
                nc.tensor.matmul(
                    aps0[:, 0:g0w],
                    lhsT=wk8_slice(r, kc),
                    rhs=ctx8_slice(0, r)[:, :, 0:g0w],
                    start=(r == 0),
                    stop=(r == 2),
                    perf_mode=mybir.MatmulPerfMode.DoubleRow,
                )
            nc.vector.tensor_copy(kt_t[kc][:, 0:g0w], aps0[:, 0:g0w])

        sc2g = {}
        for gi, (sc0_, nsc_) in enumerate(groups):
            for sc_ in range(sc0_, sc0_ + nsc_):
                sc2g[sc_] = gi

        def vprime(sc):
            """V' for one 128-wide s-chunk: [128 s, 8h*65] with mask folded."""
            g = sc2g[sc]
            off = (sc - groups[g][0]) * 128
            ps = p_a.tile([128, 512], F32, tag="a", name=f"vps{sc}")
            for c in range(6):
                nc.tensor.matmul(
                    ps[:],
                    lhsT=ctx_slice(g, c, off, off + 128),
                    rhs=wkvv_slice(c),
                    start=(c == 0),
                    stop=(c == 5),
                )
            dst = vp_t[sc][:].rearrange("p (h e) -> p h e", e=65)
            nc.vector.tensor_scalar_mul(
                dst[:, :, 0:64],
                ps[:].rearrange("p (h d) -> p h d", d=64),
                m01_t[:, sc : sc + 1],
            )
            nc.vector.tensor_scalar_mul(
                dst[:, :, 64:65],
                ones8_t[:].rearrange("p (h o) -> p h o", o=1),
                m01_t[:, sc : sc + 1],
            )

        kps_live = {}

        def kpart(g, kc, rs):
            """K-projection slice kc for ctx group g (fp8 DoubleRow passes rs)."""
            sc0, nsc = groups[g]
            w = nsc * 128
            if rs[0] == 0:
                kps_live[0] = p_a.tile([128, 512], F32, tag="a", name=f"kps{g}_{kc}")
            ps = kps_live[0]
            for r in rs:
                nc.tensor.matmul(
                    ps[:, 0:w],
                    lhsT=wk8_slice(r, kc),
                    rhs=ctx8_slice(g, r)[:, :, 0:w],
                    start=(r == 0),
                    stop=(r == 2),
                    perf_mode=mybir.MatmulPerfMode.DoubleRow,
                )
            if rs[-1] == 2:
                nc.vector.tensor_copy(
                    kt_t[kc][:, sc0 * 128 : sc0 * 128 + w], ps[:, 0:w]
                )

        def alloc_exp(g, kc):
            e0 = work.tile([128, 1024], BF16, tag="exp", bufs=8, name=f"e0_{g}_{kc}")
            e1 = work.tile([128, 1024], BF16, tag="exp", bufs=8, name=f"e1_{g}_{kc}")
            exps[(g, kc)] = (e0, e1)

        def scores_half(g, kc, jh):
            """ScoresT + exp for head pair kc, group g, j-half jh (0/1).

            Each half is a [128,<=512] psum pair from the 3-deep rotation;
            one ACT Exp per tile keeps ACT inside the PE's per-half budget.
            """
            sc0, nsc = groups[g]
            j0 = jh * 2
            j1 = min(j0 + 2, nsc)
            if j1 <= j0:
                return
            w = (j1 - j0) * 256
            e0, e1 = exps[(g, kc)]
            pe0 = p_sc.tile([128, 512], F32, tag="sc", name=f"pe0_{g}_{kc}_{jh}")
            pe1 = p_sc.tile([128, 512], F32, tag="sc", name=f"pe1_{g}_{kc}_{jh}")
            for j in range(j0, j1):
                sc = sc0 + j
                nc.tensor.matmul(
                    pe0[:, (j - j0) * 256 : (j - j0 + 1) * 256],
                    lhsT=kt_t[kc][0:64, sc * 128 : (sc + 1) * 128],
                    rhs=qt_t[kc][0:64, :],
                    start=True,
                    stop=True,
                )
                nc.tensor.matmul(
                    pe1[:, (j - j0) * 256 : (j - j0 + 1) * 256],
                    lhsT=kt_t[kc][64:128, sc * 128 : (sc + 1) * 128],
                    rhs=qt_t[kc][64:128, :],
                    start=True,
                    stop=True,
                )
            nc.scalar.activation(
                e0[:, j0 * 256 : j0 * 256 + w], pe0[:, 0:w],
                mybir.ActivationFunctionType.Exp,
            )
            nc.scalar.activation(
                e1[:, j0 * 256 : j0 * 256 + w], pe1[:, 0:w],
                mybir.ActivationFunctionType.Exp,
            )

        def pv(g, kc):
            """PV for head pair kc over group g, accumulate into pvacc."""
            sc0, nsc = groups[g]
            e0, e1 = exps.pop((g, kc))
            pvq0 = p_pv.tile([65, T], F32, tag="pv", name=f"pvq0_{g}_{kc}")
            pvq1 = p_pv.tile([65, T], F32, tag="pv", name=f"pvq1_{g}_{kc}")
            for j in range(nsc):
                sc = sc0 + j
                nc.tensor.matmul(
                    pvq0[:],
                    lhsT=vp_t[sc][:, (2 * kc) * 65 : (2 * kc) * 65 + 65],
                    rhs=e0[:, j * 256 : (j + 1) * 256],
                    start=(j == 0),
                    stop=(j == nsc - 1),
                )
                nc.tensor.matmul(
                    pvq1[:],
                    lhsT=vp_t[sc][:, (2 * kc + 1) * 65 : (2 * kc + 1) * 65 + 65],
                    rhs=e1[:, j * 256 : (j + 1) * 256],
                    start=(j == 0),
                    stop=(j == nsc - 1),
                )
            if g == 0:
                nc.vector.tensor_copy(pvacc_t[2 * kc][:], pvq0[:])
                nc.vector.tensor_copy(pvacc_t[2 * kc + 1][:], pvq1[:])
            else:
                nc.vector.tensor_add(pvacc_t[2 * kc][:], pvacc_t[2 * kc][:], pvq0[:])
                nc.vector.tensor_add(
                    pvacc_t[2 * kc + 1][:], pvacc_t[2 * kc + 1][:], pvq1[:]
                )

        exps = {}

        def den_chain(kc):
            """Collect denominators of head pair kc, reciprocal, refold to rech."""
            nc.sync.dma_start(den2_t[kc][0:1, :], pvacc_t[2 * kc][64:65, :])
            nc.gpsimd.dma_start(den2_t[kc][1:2, :], pvacc_t[2 * kc + 1][64:65, :])
            nc.vector.reciprocal_approx_fast(rec2_t[kc][:], den2_t[kc][:])
            nc.sync.dma_start(
                rech_t[0:1, (2 * kc) * T : (2 * kc + 2) * T].rearrange(
                    "p (h t) -> p h t", t=T
                ),
                rec2_t[kc][:],
            )

        def bc_mul(kc):
            """Broadcast 1/den over 64 partitions (K=1 matmul), scale PV -> OT."""
            bc = p_a.tile([128, 512], F32, tag="a", name=f"bc{kc}")
            nc.tensor.matmul(
                bc[0:64, 0:512],
                lhsT=ones64_t[:],
                rhs=rech_t[0:1, (2 * kc) * T : (2 * kc + 2) * T],
                start=True,
                stop=True,
            )
            tmp1 = work.tile([64, T], BF16, tag="otmp", bufs=2, name=f"otmp{kc}")
            nc.vector.tensor_mul(
                tmp1[:], pvacc_t[2 * kc + 1][0:64, :], bc[0:64, T : 2 * T]
            )
            nc.gpsimd.dma_start(ot_t[kc][64:128, :], tmp1[:])
            nc.vector.tensor_mul(
                ot_t[kc][0:64, :], pvacc_t[2 * kc][0:64, :], bc[0:64, 0:T]
            )

        def outproj_all():
            """Out-proj eo-outer: one [128,512] accumulator at a time from
            the 3-deep rotation; bias + output DMA fire per eo."""
            for eo in range(4):
                reg = p_sc.tile([128, 512], F32, tag="sc", name=f"ops{eo}")
                for kc in range(4):
                    nc.tensor.matmul(
                        reg[:, 0:T],
                        lhsT=wop_t[:, kc * 512 + eo * 128 : kc * 512 + (eo + 1) * 128],
                        rhs=ot_t[kc][:],
                        start=(kc == 0),
                        stop=(kc == 3),
                    )
                osb = work.tile([128, T], F32, tag="osb", bufs=4, name=f"osb{eo}")
                nc.vector.tensor_scalar_add(osb[:], reg[:, 0:T], bo_t[:, eo : eo + 1])
                q = nc.sync if eo % 2 == 0 else nc.gpsimd
                q.dma_start(outT_d[eo], osb[:])

        # ---- V'(0) then pipelined groups -----------------------------------
        for sc in range(groups[0][0], groups[0][0] + groups[0][1]):
            vprime(sc)

        # Single-phase slots: scores/kpart/V'/PV interleaved per head pair
        # (PV shifted one slot so its exp is ready) keeps every engine fed
        # at its own rate instead of alternating ACT-paced and ACT-idle
        # phases.
        for g in range(1, n_g):
            for kc in range(4):
                alloc_exp(g - 1, kc)
                scores_half(g - 1, kc, 0)
                kpart(g, kc, [0, 1])
                scores_half(g - 1, kc, 1)
                kpart(g, kc, [2])
                if kc < groups[g][1]:
                    vprime(groups[g][0] + kc)
                if kc >= 1:
                    pv(g - 1, kc - 1)
            pv(g - 1, 3)

        # ---- final group: scores/PV interleaved with the normalization tail.
        gl = n_g - 1
        for kc in range(4):
            alloc_exp(gl, kc)
            scores_half(gl, kc, 0)
            if kc >= 1:
                pv(gl, kc - 1)
                den_chain(kc - 1)
            scores_half(gl, kc, 1)
            if kc >= 2:
                bc_mul(kc - 2)
        pv(gl, 3)
        den_chain(3)
        bc_mul(2)
        bc_mul(3)
        outproj_all()

    nc.compile()
    return nc


_NC_CACHE = {}


def _get_nc(s_pad):
    if s_pad not in _NC_CACHE:
        _NC_CACHE[s_pad] = _build_program(s_pad)
    return _NC_CACHE[s_pad]


def _pack6(mT):
    """[k*128, N] -> [128, k*N] with chunk c at cols c*N."""
    n = mT.shape[1]
    return np.ascontiguousarray(
        mT.reshape(-1, 128, n).transpose(1, 0, 2).reshape(128, -1)
    )


def _prep_in_maps(x, context, key_padding_mask, Wq, Wkv, Wo, bo):
    keep = [np.flatnonzero(~key_padding_mask[b]) for b in range(B)]
    max_keep = max(len(k) for k in keep)
    s_pad = max(128, -(-max_keep // 128) * 128)
    groups = _groups(s_pad // 128)

    wqp = _pack6((np.ascontiguousarray(Wq.T) * np.float32(D**-0.5)).astype(NPBF16))
    wkvT = np.ascontiguousarray(Wkv.T)  # [768, 1024]
    wkvp = _pack6(np.ascontiguousarray(wkvT[:, 512:1024]).astype(NPBF16))
    wk8p = np.ascontiguousarray(
        wkvT[:, 0:512].reshape(3, 2, 128, 512).transpose(2, 0, 1, 3).reshape(128, -1)
    ).astype(NPF8)
    wop = _pack6(np.ascontiguousarray(Wo.T).astype(NPBF16))
    bo_r = np.ascontiguousarray(bo.reshape(4, 128).T).astype(np.float32)
    in_maps = []
    for b in range(B):
        nk = len(keep[b])
        ctxc = np.zeros((s_pad, KV), dtype=np.float32)
        ctxc[:nk] = context[b][keep[b]]
        ctxT = np.ascontiguousarray(ctxc.T).astype(NPBF16)  # [768, s_pad]
        ctx3 = ctxT.reshape(6, 128, s_pad)
        m = np.zeros(s_pad, dtype=np.float32)
        m[:nk] = 1.0
        m01 = np.ascontiguousarray(m.reshape(s_pad // 128, 128).T)
        ctx8 = np.ascontiguousarray(
            ctxT.astype(np.float32)
            .reshape(3, 2, 128, s_pad)
            .transpose(2, 0, 1, 3)
        ).astype(NPF8)  # [128, 3, 2, s_pad]
        im = dict(
            xp=_pack6(np.ascontiguousarray(x[b].T).astype(NPBF16)),
            m01=m01, wqp=wqp, wkvp=wkvp, wk8p=wk8p, wop=wop, bo_r=bo_r,
        )
        for g, (sc0, nsc) in enumerate(groups):
            blk = ctx3[:, :, sc0 * 128 : (sc0 + nsc) * 128]  # [6,128,w]
            im[f"ctxp{g}"] = np.ascontiguousarray(
                blk.transpose(1, 0, 2).reshape(128, -1)
            )
            im[f"ctx8p{g}"] = np.ascontiguousarray(
                ctx8[:, :, :, sc0 * 128 : (sc0 + nsc) * 128].reshape(128, -1)
            )
        in_maps.append(im)
    return in_maps, s_pad


def _run(inputs, trace=False, **kw):
    in_maps, s_pad = _prep_in_maps(**inputs)
    nc = _get_nc(s_pad)
    res = bass_utils.run_bass_kernel_spmd(
        nc, in_maps, core_ids=list(range(NC_CORES)), trace=trace, **kw
    )
    out = np.stack(
        [res.results[b]["outT"].reshape(E, T).T for b in range(B)]
    ).astype(np.float32)
    return out, res


def kernel(**inputs):
    out, _ = _run(inputs, trace=False)
    return out


if __name__ == "__main__":
    rng = np.random.default_rng(0)
    ins = dict(
        x=rng.standard_normal((B, T, E), dtype=np.float32),
        context=rng.standard_normal((B, S, KV), dtype=np.float32),
        key_padding_mask=rng.integers(0, 2, (B, S)).astype(bool),
        Wq=(rng.standard_normal((512, E), dtype=np.float32) * 0.02),
        Wkv=(rng.standard_normal((1024, KV), dtype=np.float32) * 0.02),
        Wo=(rng.standard_normal((E, 512), dtype=np.float32) * 0.02),
        bo=np.zeros(E, dtype=np.float32),
    )
    out = kernel(**ins)
    print("out", out.shape, out.dtype, np.abs(out).mean())
